# revision 13
# baseline (speedup 1.0000x reference)
"""Llama decode attention (paged KV, GQA) as a Bass/Tile kernel on 8 TRN2 cores.

Sharding: tensor-parallel by kv-head. Core c owns q heads 4c..4c+3, kv head c,
the matching W_qkv column shard, that kv-head's slice of the paged KV cache,
and the W_o row shard. Each core computes a partial [32, 4096] output; the
host sums the 8 partials (the "all-reduce") and adds b_o.

Host-side staging builds matmul-native KV layouts per core:
  - K: [128 (head dim), TOT*128 (chunk-major tokens)]  -> score matmul lhsT
  - V: [128 (token-in-chunk), TOT*129]; per chunk cols 0:128 = V rows,
    col 128 = validity (1.0 valid / 0.0 pad-or-new-token-slot), 129:132 pad.
    The validity column doubles as the softmax-denominator accumulator via a
    [tokens,1] x [tokens,4] matmul, so no masking ops are needed on device.
The new token's k/v (computed in-kernel from the QKV projection) enter
attention through one extra 32-token "chunk" (kt_new / vnew) with a
block-diagonal probability mask, so nothing is inserted into the KV tiles.

Schedule: the kernel is DMA-bound (~24 MB/core at ~370 GB/s). DMA order is
arranged so nothing steals bandwidth from the critical path:
  ht -> wq (8 slices, alternating HWDGE rings; QKV matmuls chase them
  ki-by-ki) -> K/V windows (K and V of each window on opposite rings to
  balance ring bytes) -> wo lands just before the last two windows ->
  fp16 output stores.
"""

import math

import numpy as np
import ml_dtypes

H = 32
KVH = 8
D = 128
HIDDEN = 4096
Q_SIZE = H * D
KV_SIZE = KVH * D
BLOCK = 16
NBLOCKS = 8192
MAXBPS = 128
MAXCTX = 2048
B = 32
NCORES = 8
GQ = H // NCORES          # q heads per core = 4
CHUNK = 128               # tokens per chunk
BPC = CHUNK // BLOCK      # blocks per chunk = 8
VW = 129                  # merged V chunk width: 128 D + 1 validity
VBF = 48                  # V head-dims kept in bf16 (rest fp8); K is all fp8
VLW = D - VBF + 1         # fp8 V slice width: 80 lo dims + validity = 81
WCH = 48                  # chunks per DMA window (max; tail windows taper)
WAVE = 16                 # chunks per exp wave
ROPE_THETA = 10000.0
SCALE = D ** -0.5
EXP_BIAS = -2.0           # exp(s*SCALE - 2): headroom vs overflow, cancels in norm

BF16 = ml_dtypes.bfloat16
FP16 = np.float16
FP8KV = ml_dtypes.float8_e3m4


def _ceil_div(a, b):
    return -(-a // b)


def _window_sizes(tot):
    """Full windows of WCH chunks; split the remainder so the final window is
    small (short post-stream PE tail)."""
    sizes = []
    rem = tot
    while rem > WCH:
        sizes.append(WCH)
        rem -= WCH
    if rem > 12:
        sizes.extend([rem - 12, 12])
    else:
        sizes.append(rem)
    return sizes


class _Schedule:
    """Static per-call schedule derived from context_lens/block_tables."""

    def __init__(self, context_lens, block_tables):
        ctx = np.asarray(context_lens, np.int64)
        bt = np.asarray(block_tables, np.int64)
        self.ctx = ctx
        self.bt = bt
        self.pos = ctx - 1
        self.nch = np.maximum(1, _ceil_div(ctx, CHUNK)).astype(np.int64)
        self.tot = int(self.nch.sum())
        self.chunk_seq = np.repeat(np.arange(B), self.nch)        # [tot]
        ci = np.concatenate([np.arange(n) for n in self.nch])
        self.chunk_ci = ci                                        # [tot]

        # RoPE tables at the new-token position
        half = D // 2
        inv_freq = 1.0 / (ROPE_THETA ** (np.arange(half, dtype=np.float64) / half))
        ang = self.pos[:, None].astype(np.float64) * inv_freq[None, :]
        self.cosf = np.tile(np.cos(ang).astype(np.float32), (1, 5))  # [32, 320]
        self.sinf = np.tile(np.sin(ang).astype(np.float32), (1, 5))

        # block-diagonal probability mask for the new-token chunk
        md = np.zeros((B, GQ * B), np.float32)
        for b in range(B):
            md[b, GQ * b:GQ * (b + 1)] = 1.0
        self.mdiag = md.astype(BF16)

        # per-chunk token validity [tot, 128]: g < ctx and g != pos
        g = ci[:, None] * CHUNK + np.arange(CHUNK)[None, :]
        s = self.chunk_seq[:, None]
        self.valid = ((g < ctx[s]) & (g != self.pos[s])).astype(np.float32)

        # flat gathered block list [tot*8]
        blk = []
        for b in range(B):
            blk.append(bt[b, :self.nch[b] * BPC])
        self.blocks_flat = np.concatenate(blk)


def _emit(nc, tile, mybir, sched):
    """Emit the per-core kernel (same NEFF for all cores)."""
    from concourse.masks import make_identity
    from concourse.tile import add_dep_helper

    dt = mybir.dt
    sc = sched
    TOT = sc.tot
    wsizes = _window_sizes(TOT)
    NWIN = len(wsizes)
    wstart = np.concatenate([[0], np.cumsum(wsizes)]).astype(int)

    # ---- DRAM I/O ----
    d_ht = nc.dram_tensor("ht", [128, 32, B], dt.bfloat16, kind="ExternalInput")
    d_wq = nc.dram_tensor("wq", [128, 32, 768], dt.bfloat16, kind="ExternalInput")
    d_wo = nc.dram_tensor("wo", [128, 4, HIDDEN], dt.bfloat16, kind="ExternalInput")
    d_bq = nc.dram_tensor("bq", [1, 768], dt.bfloat16, kind="ExternalInput")
    d_trig = nc.dram_tensor("trig", [B, 640], dt.float32, kind="ExternalInput")
    d_md = nc.dram_tensor("mdiag", [B, GQ * B], dt.bfloat16, kind="ExternalInput")
    d_kg = nc.dram_tensor("kg", [128, TOT * CHUNK], dt.bfloat16, kind="ExternalInput")
    d_vg = nc.dram_tensor("vg", [128, TOT * VW], dt.float8e3, kind="ExternalInput")
    d_out = nc.dram_tensor("out", [128, B * HIDDEN // 128], dt.float16,
                           kind="ExternalOutput")

    with tile.TileContext(nc) as tc:
        with (
            tc.tile_pool(name="const", bufs=1) as cp,
            tc.tile_pool(name="work", bufs=1) as wp,
            tc.tile_pool(name="kwp", bufs=4) as kwp,
            tc.tile_pool(name="vwp", bufs=4) as vwp,
            tc.tile_pool(name="extp", bufs=4) as extp,
            tc.tile_pool(name="pswork", bufs=1, space="PSUM") as pswork,
            tc.tile_pool(name="pssc", bufs=2, space="PSUM") as pssc,
            tc.tile_pool(name="psacc", bufs=1, space="PSUM") as psacc,
        ):
            # ---- critical-path DMAs first: ht then wq slices, alternating
            # between the two HWDGE rings so both drain the weights together.
            ht = cp.tile([128, 32, B], dt.bfloat16, tag="ht")
            nc.sync.dma_start(ht[:], d_ht[:])
            bq = cp.tile([1, 768], dt.bfloat16, tag="bq")
            nc.scalar.dma_start(bq[:], d_bq[:])
            trig = cp.tile([B, 640], dt.float32, tag="trig")
            nc.scalar.dma_start(trig[:], d_trig[:])
            mdiag = cp.tile([B, GQ * B], dt.bfloat16, tag="mdiag")
            nc.scalar.dma_start(mdiag[:], d_md[:])
            # wq: q-part (cols 0:512) first — it alone gates the attention
            # start; the k/v part (cols 512:768) is only needed by the
            # new-token path and lands after window 0.
            wq = cp.tile([128, 32, 768], dt.bfloat16, tag="wq")
            for qd in range(4):
                eng = nc.sync if qd % 2 == 0 else nc.scalar
                eng.dma_start(wq[:, 8 * qd:8 * (qd + 1), 0:512],
                              d_wq[:, 8 * qd:8 * (qd + 1), 0:512])
            cosf = trig[:, 0:320]
            sinf = trig[:, 320:640]
            wo = cp.tile([128, 4, HIDDEN], dt.bfloat16, tag="wo")

            zrow = cp.tile([128, 384], dt.bfloat16, tag="zrow")
            nc.vector.memset(zrow[:], 0.0)
            ones1 = cp.tile([1, 128], dt.bfloat16, tag="ones1")
            nc.vector.memset(ones1[:], 1.0)
            ones32 = cp.tile([32, 1], dt.bfloat16, tag="ones32")
            nc.vector.memset(ones32[:], 1.0)
            ident = cp.tile([128, 128], dt.bfloat16, tag="ident")
            make_identity(nc, ident[:])
            ebias = cp.tile([128, 1], dt.float32, tag="ebias")
            nc.vector.memset(ebias[:], EXP_BIAS)

            # ---- KV windows on alternating rings (K of window w and V of
            # window w drain in parallel); window 0 is emitted before the
            # wq k/v-part so attention can start as early as possible; wo
            # rides both rings after window 2.
            kdmas = []
            wins = []
            for w in range(NWIN):
                c0 = int(wstart[w])
                c1 = int(wstart[w + 1])
                wsz = c1 - c0
                keng = nc.sync if w % 2 == 0 else nc.scalar
                veng = nc.scalar if w % 2 == 0 else nc.sync
                kwin = kwp.tile([128, CHUNK * WCH], dt.bfloat16, tag="kw")
                kd = keng.dma_start(kwin[:, :CHUNK * wsz],
                                    d_kg[:, CHUNK * c0:CHUNK * c1])
                kdmas.append(kd)
                vwin = vwp.tile([128, VW * WCH], dt.float8e3, tag="vw")
                veng.dma_start(vwin[:, :VW * wsz],
                               d_vg[:, VW * c0:VW * c1])
                wins.append((kwin, vwin, c0, c1))
                if w == 0:
                    for qd in range(2):
                        eng = nc.sync if qd == 0 else nc.scalar
                        eng.dma_start(wq[:, 16 * qd:16 * (qd + 1), 512:768],
                                      d_wq[:, 16 * qd:16 * (qd + 1), 512:768])
                if w == 2:
                    for wn in range(2):
                        eng = nc.sync if wn == 0 else nc.scalar
                        eng.dma_start(wo[:, :, 2048 * wn:2048 * (wn + 1)],
                                      d_wo[:, :, 2048 * wn:2048 * (wn + 1)])

            # ---- PE warm-up fills (cover the pre-wq DMA latency only) ----
            wu = pssc.tile([32, 384], dt.float32, tag="sc")
            for i in range(12):
                nc.tensor.matmul(wu[:], lhsT=zrow[:, 0:32], rhs=zrow[:],
                                 start=True, stop=True, skip_group_check=True)

            # ---- QKV projection, q-part first: q[32, 512] = hT.T @ wq_q ----
            ps_qa = pswork.tile([B, 512], dt.float32, tag="mm")
            nc.tensor.matmul(ps_qa[:], lhsT=ones1[0:1, 0:B], rhs=bq[0:1, 0:512],
                             start=True, stop=False)
            for ki in range(32):
                nc.tensor.matmul(ps_qa[:], lhsT=ht[:, ki, :],
                                 rhs=wq[:, ki, 0:512], start=False, stop=ki == 31)
            qkv_f = wp.tile([B, 768], dt.float32, tag="qkvf")
            nc.vector.tensor_copy(qkv_f[:, 0:512], ps_qa[:])

            # ---- RoPE on the 4 q heads (rotate-half on the free axis) ----
            qk_rope = wp.tile([B, 768], dt.bfloat16, tag="qkrope")

            def rope(h0, h1):
                n = h1 - h0
                qs = qkv_f[:, 128 * h0:128 * h1].rearrange(
                    "p (h t x) -> p h t x", t=2, x=64)
                rs = qk_rope[:, 128 * h0:128 * h1].rearrange(
                    "p (h t x) -> p h t x", t=2, x=64)
                ch = c5[:, h0:h1, :]
                sh = s5[:, h0:h1, :]
                t1 = wp.tile([B, 5, 64], dt.float32, tag="t1")
                t2 = wp.tile([B, 5, 64], dt.float32, tag="t2")
                nc.vector.tensor_mul(t1[:, :n], qs[:, :, 0, :], ch)
                nc.vector.tensor_mul(t2[:, :n], qs[:, :, 1, :], sh)
                nc.vector.tensor_sub(rs[:, :, 0, :], t1[:, :n], t2[:, :n])
                t3 = wp.tile([B, 5, 64], dt.float32, tag="t1")
                t4 = wp.tile([B, 5, 64], dt.float32, tag="t2")
                nc.vector.tensor_mul(t3[:, :n], qs[:, :, 1, :], ch)
                nc.vector.tensor_mul(t4[:, :n], qs[:, :, 0, :], sh)
                nc.vector.tensor_add(rs[:, :, 1, :], t3[:, :n], t4[:, :n])

            c5 = cosf.rearrange("p (h x) -> p h x", x=64)
            s5 = sinf.rearrange("p (h x) -> p h x", x=64)
            rope(0, 4)

            # ---- transpose q heads: qt [128, 4b+h] ----
            qt = wp.tile([128, GQ * B], dt.bfloat16, tag="qt")
            kt_new = wp.tile([128, B], dt.bfloat16, tag="ktnew")
            for hh in range(4):
                pst = pswork.tile([128, B], dt.bfloat16, tag="tr", bufs=3)
                nc.tensor.transpose(
                    pst[:], qk_rope[:, 128 * hh:128 * (hh + 1)], ident[:B, :B]
                )
                nc.vector.tensor_copy(qt[:, hh::4], pst[:])

            vnew = wp.tile([B, 128], dt.bfloat16, tag="vnew")
            p2 = wp.tile([B, 128], dt.bfloat16, tag="p2")

            def emit_kv_path():
                # k/v part of the projection + new-token probabilities.
                # Emitted mid-wave-stream (in PE program order) so these
                # matmuls don't sit in the PE FIFO ahead of the first waves
                # while waiting on the late wq k/v slice.
                ps_qb = pswork.tile([B, 256], dt.float32, tag="qk1")
                nc.tensor.matmul(ps_qb[:], lhsT=ones1[0:1, 0:B],
                                 rhs=bq[0:1, 512:768], start=True, stop=False)
                for ki in range(32):
                    nc.tensor.matmul(ps_qb[:], lhsT=ht[:, ki, :],
                                     rhs=wq[:, ki, 512:768], start=False,
                                     stop=ki == 31)
                nc.vector.tensor_copy(qkv_f[:, 512:768], ps_qb[:])
                rope(4, 5)
                nc.vector.tensor_copy(vnew[:], qkv_f[:, 640:768])
                pst = pswork.tile([128, B], dt.bfloat16, tag="tr", bufs=3)
                nc.tensor.transpose(pst[:], qk_rope[:, 512:640], ident[:B, :B])
                nc.vector.tensor_copy(kt_new[:], pst[:])
                ps_x = pswork.tile([B, 128], dt.float32, tag="mm")
                nc.tensor.matmul(ps_x[:], lhsT=kt_new[:], rhs=qt[:],
                                 start=True, stop=True)
                extx = wp.tile([B, 128], dt.float32, tag="extx")
                nc.scalar.activation(
                    extx[:], ps_x[:], mybir.ActivationFunctionType.Exp,
                    bias=ebias[0:B, :], scale=SCALE,
                )
                nc.vector.tensor_mul(p2[:], extx[:], mdiag[:])

            # ---- zero the attention accumulator (data=0, defined has_written) ----
            ps_acc = psacc.tile([128, 256], dt.float32, tag="acc")
            nc.tensor.matmul(ps_acc[:, 0:256],
                             lhsT=zrow[:, 0:128], rhs=zrow[:, 0:256],
                             start=True, stop=False, skip_group_check=True)

            # ---- waves: software-pipelined one deep ----
            waves = []
            for kwin, vwin, c0, c1 in wins:
                for ws in range(c0, c1, WAVE):
                    waves.append((kwin, vwin, c0, ws, min(WAVE, c1 - ws)))

            exts = {}

            def emit_scores(i):
                kwin, vwin, c0, ws, n = waves[i]
                ps_sc = pssc.tile([128, 4 * WAVE], dt.float32, tag="sc",
                                  name=f"pssc{i}")
                for j in range(n):
                    ch = ws + j
                    l = ch - c0
                    b = int(sc.chunk_seq[ch])
                    nc.tensor.matmul(
                        ps_sc[:, 4 * j:4 * (j + 1)],
                        lhsT=kwin[:, CHUNK * l:CHUNK * (l + 1)],
                        rhs=qt[:, GQ * b:GQ * (b + 1)],
                        start=True, stop=True,
                    )
                ext = extp.tile([128, 4 * WAVE], dt.bfloat16, tag="ext",
                                name=f"ext{i}")
                nc.scalar.activation(
                    ext[:, :4 * n], ps_sc[:, :4 * n],
                    mybir.ActivationFunctionType.Exp,
                    bias=ebias[:], scale=SCALE,
                )
                exts[i] = ext

            emit_scores(0)
            for i in range(len(waves)):
                if i == 2:
                    emit_kv_path()
                if i + 1 < len(waves):
                    emit_scores(i + 1)  # PE does wave i+1 scores while exp(i) runs
                kwin, vwin, c0, ws, n = waves[i]
                ext = exts.pop(i)
                # uniform runs so the PE drain/fill overlap never breaks
                for j in range(n):
                    ch = ws + j
                    l = ch - c0
                    b = int(sc.chunk_seq[ch])
                    nc.tensor.matmul(
                        ps_acc[:, 4 * b:4 * (b + 1)],
                        lhsT=vwin[:, VW * l:VW * l + 128],
                        rhs=ext[:, 4 * j:4 * (j + 1)],
                        start=False, stop=False, skip_group_check=True,
                    )
                for j in range(n):
                    ch = ws + j
                    l = ch - c0
                    b = int(sc.chunk_seq[ch])
                    nc.tensor.matmul(
                        ps_acc[0:1, 128 + 4 * b:132 + 4 * b],
                        lhsT=vwin[:, VW * l + 128:VW * l + 129],
                        rhs=ext[:, 4 * j:4 * (j + 1)],
                        start=False, stop=False, skip_group_check=True,
                    )

            # ---- new-token contribution (precomputed p2 above) ----
            nc.tensor.matmul(ps_acc[:, 0:128], lhsT=vnew[:], rhs=p2[:],
                             start=False, stop=True, skip_group_check=True)
            nc.tensor.matmul(ps_acc[0:1, 128:256], lhsT=ones32[:], rhs=p2[:],
                             start=False, stop=True, skip_group_check=True)

            # ---- normalize: at = attn / denom ----
            # broadcast the sums across partitions FIRST, then a
            # 128-partition-parallel reciprocal (serial chain ~2x shorter)
            sums_bf = wp.tile([1, 128], dt.bfloat16, tag="sumsbf")
            nc.vector.tensor_copy(sums_bf[:], ps_acc[0:1, 128:256])
            ps_rb = pswork.tile([128, 128], dt.float32, tag="mm")
            nc.tensor.matmul(ps_rb[:], lhsT=ones1[0:1, :], rhs=sums_bf[0:1, :],
                             start=True, stop=True)
            rb_sb = wp.tile([128, 128], dt.float32, tag="rbsb")
            nc.vector.reciprocal(rb_sb[:], ps_rb[:])
            # head-major layout: at_hm[:, 32*h + s] = attn[:, 4*s + h]
            at_hm = wp.tile([128, 128], dt.bfloat16, tag="athm")
            nc.vector.tensor_mul(
                at_hm[:].rearrange("p (h s) -> p s h", h=4),
                ps_acc[:, 0:128].rearrange("p (s h) -> p s h", h=4),
                rb_sb[:].rearrange("p (s h) -> p s h", h=4),
            )

            # ---- O projection, transposed: outT[4096, 32] partial ----
            # lhsT = wo[:, h, 128j:128j+128] ([d, n] block), rhs = at_hm head
            # slice ([d, s]); psum groups of 4 j-blocks -> ostage [128, 1024]
            ostage = wp.tile([128, 32 * 32], dt.float16, tag="ostage")
            for g in range(8):
                ps_o = pswork.tile([128, 128], dt.float32, tag="tr", bufs=3)
                for jj in range(4):
                    j = 4 * g + jj
                    for hh in range(4):
                        nc.tensor.matmul(
                            ps_o[:, 32 * jj:32 * (jj + 1)],
                            lhsT=wo[:, hh, 128 * j:128 * (j + 1)],
                            rhs=at_hm[:, 32 * hh:32 * (hh + 1)],
                            start=(hh == 0),
                            stop=(hh == 3),
                            skip_group_check=True,
                        )
                if g % 2 == 0:
                    nc.scalar.copy(ostage[:, 128 * g:128 * (g + 1)], ps_o[:])
                else:
                    nc.vector.tensor_copy(ostage[:, 128 * g:128 * (g + 1)], ps_o[:])
                if g % 4 == 3:
                    nc.sync.dma_start(d_out[:, 128 * (g - 3):128 * (g + 1)],
                                      ostage[:, 128 * (g - 3):128 * (g + 1)])

    nc.compile()
    return nc


def _build_inputs(sched, hidden_states, W_qkv, b_qkv, W_o, k_cache, v_cache):
    """Per-core input maps with host-side gather into matmul-native layouts."""
    sc = sched
    TOT = sc.tot

    hts = hidden_states.T.astype(BF16)  # [4096, 32]
    ht_in = np.ascontiguousarray(hts.reshape(32, 128, B).transpose(1, 0, 2))

    # one global gather of the needed blocks (all kv heads at once)
    KB = k_cache[sc.blocks_flat]   # [TOT*8, 16, 8, 128] fp32
    VB = v_cache[sc.blocks_flat]

    maps = []
    for c in range(NCORES):
        qr = slice(512 * c, 512 * (c + 1))
        kr = slice(Q_SIZE + 128 * c, Q_SIZE + 128 * (c + 1))
        vr = slice(Q_SIZE + KV_SIZE + 128 * c, Q_SIZE + KV_SIZE + 128 * (c + 1))
        wq_sh = np.concatenate([W_qkv[qr], W_qkv[kr], W_qkv[vr]], axis=0)  # [768, 4096]
        wq_in = np.ascontiguousarray(
            wq_sh.T.astype(BF16).reshape(32, 128, 768).transpose(1, 0, 2))
        bq_sh = np.concatenate([b_qkv[qr], b_qkv[kr], b_qkv[vr]])
        bq_in = bq_sh[None, :].astype(BF16)
        wo_in = np.ascontiguousarray(
            W_o[:, qr].T.astype(BF16).reshape(4, 128, HIDDEN).transpose(1, 0, 2))

        # K: [TOT, 128 tok, 128 D] -> [128 D, TOT*128]
        kc = KB[:, :, c, :].astype(BF16).reshape(TOT, CHUNK, D)
        kg_in = np.ascontiguousarray(
            kc.transpose(2, 0, 1).reshape(D, TOT * CHUNK))

        # V: [TOT, 128 tok, 129]
        vc = VB[:, :, c, :].reshape(TOT, CHUNK, D)
        vg = np.zeros((TOT, CHUNK, VW), np.float32)
        vg[:, :, :D] = vc * sc.valid[:, :, None]
        vg[:, :, D] = sc.valid
        vg_in = np.ascontiguousarray(
            vg.astype(FP8KV).transpose(1, 0, 2).reshape(CHUNK, TOT * VW))

        maps.append({
            "ht": ht_in, "wq": wq_in, "wo": wo_in, "bq": bq_in,
            "trig": np.concatenate([sc.cosf, sc.sinf], axis=1),
            "mdiag": sc.mdiag,
            "kg": kg_in, "vg": vg_in,
        })
    return maps


_TRACE = {"on": False, "result": None}


def kernel(hidden_states, W_qkv, b_qkv, W_o, b_o, k_cache, v_cache,
           block_tables, context_lens):
    import concourse.tile as tile
    import concourse.mybir as mybir
    from concourse import bacc
    from concourse.bass_utils import run_bass_kernel_spmd

    sched = _Schedule(context_lens, block_tables)
    nc = bacc.Bacc("TRN2", target_bir_lowering=False, debug=False)
    _emit(nc, tile, mybir, sched)

    in_maps = _build_inputs(sched, np.asarray(hidden_states, np.float32),
                            np.asarray(W_qkv, np.float32),
                            np.asarray(b_qkv, np.float32),
                            np.asarray(W_o, np.float32),
                            np.asarray(k_cache, np.float32),
                            np.asarray(v_cache, np.float32))

    res = run_bass_kernel_spmd(nc, in_maps, core_ids=list(range(NCORES)),
                               trace=_TRACE["on"])
    _TRACE["result"] = res

    acc = np.zeros((B, HIDDEN), np.float64)
    for c in range(NCORES):
        o128 = res.results[c]["out"].astype(np.float64)  # [128, 1024]
        # o128[p, 32*j + s] = out[s, 128*j + p]
        acc += o128.reshape(128, 32, 32).transpose(2, 1, 0).reshape(B, HIDDEN)
    acc += np.asarray(b_o, np.float64)[None, :]
    return acc.astype(np.float32)


# revision 21
# speedup vs baseline: 1.0037x; 1.0037x over previous
"""Llama decode attention (paged KV, GQA) as a Bass/Tile kernel on 8 TRN2 cores.

Sharding: tensor-parallel by kv-head. Core c owns q heads 4c..4c+3, kv head c,
the matching W_qkv column shard, that kv-head's slice of the paged KV cache,
and the W_o row shard. Each core computes a partial [32, 4096] output; the
host sums the 8 partials (the "all-reduce") and adds b_o.

Host-side staging builds matmul-native KV layouts per core:
  - K: [128 (head dim), TOT*128 (chunk-major tokens)]  -> score matmul lhsT
  - V: [128 (token-in-chunk), TOT*129]; per chunk cols 0:128 = V rows,
    col 128 = validity (1.0 valid / 0.0 pad-or-new-token-slot), 129:132 pad.
    The validity column doubles as the softmax-denominator accumulator via a
    [tokens,1] x [tokens,4] matmul, so no masking ops are needed on device.
The new token's k/v (computed in-kernel from the QKV projection) enter
attention through one extra 32-token "chunk" (kt_new / vnew) with a
block-diagonal probability mask, so nothing is inserted into the KV tiles.

Schedule: the kernel is DMA-bound (~24 MB/core at ~370 GB/s). DMA order is
arranged so nothing steals bandwidth from the critical path:
  ht -> wq (8 slices, alternating HWDGE rings; QKV matmuls chase them
  ki-by-ki) -> K/V windows (K and V of each window on opposite rings to
  balance ring bytes) -> wo lands just before the last two windows ->
  fp16 output stores.
"""

import math

import numpy as np
import ml_dtypes

H = 32
KVH = 8
D = 128
HIDDEN = 4096
Q_SIZE = H * D
KV_SIZE = KVH * D
BLOCK = 16
NBLOCKS = 8192
MAXBPS = 128
MAXCTX = 2048
B = 32
NCORES = 8
GQ = H // NCORES          # q heads per core = 4
CHUNK = 128               # tokens per chunk
BPC = CHUNK // BLOCK      # blocks per chunk = 8
VW = 129                  # merged V chunk width: 128 D + 1 validity
VBF = 48                  # V head-dims kept in bf16 (rest fp8); K is all fp8
VLW = D - VBF + 1         # fp8 V slice width: 80 lo dims + validity = 81
WCH = 48                  # chunks per DMA window (max; tail windows taper)
WAVE = 16                 # chunks per exp wave
ROPE_THETA = 10000.0
SCALE = D ** -0.5
EXP_BIAS = -2.0           # exp(s*SCALE - 2): headroom vs overflow, cancels in norm

BF16 = ml_dtypes.bfloat16
FP16 = np.float16
FP8KV = ml_dtypes.float8_e3m4


def _ceil_div(a, b):
    return -(-a // b)


def _window_sizes(tot):
    """Full windows of WCH chunks; split the remainder so the final window is
    small (short post-stream PE tail)."""
    sizes = []
    rem = tot
    while rem > WCH:
        sizes.append(WCH)
        rem -= WCH
    if rem > 12:
        sizes.extend([rem - 12, 12])
    else:
        sizes.append(rem)
    return sizes


class _Schedule:
    """Static per-call schedule derived from context_lens/block_tables."""

    def __init__(self, context_lens, block_tables):
        ctx = np.asarray(context_lens, np.int64)
        bt = np.asarray(block_tables, np.int64)
        self.ctx = ctx
        self.bt = bt
        self.pos = ctx - 1
        self.nch = np.maximum(1, _ceil_div(ctx, CHUNK)).astype(np.int64)
        self.tot = int(self.nch.sum())
        self.chunk_seq = np.repeat(np.arange(B), self.nch)        # [tot]
        ci = np.concatenate([np.arange(n) for n in self.nch])
        self.chunk_ci = ci                                        # [tot]

        # RoPE tables at the new-token position
        half = D // 2
        inv_freq = 1.0 / (ROPE_THETA ** (np.arange(half, dtype=np.float64) / half))
        ang = self.pos[:, None].astype(np.float64) * inv_freq[None, :]
        self.cosf = np.tile(np.cos(ang).astype(np.float32), (1, 5))  # [32, 320]
        self.sinf = np.tile(np.sin(ang).astype(np.float32), (1, 5))

        # block-diagonal probability mask for the new-token chunk
        md = np.zeros((B, GQ * B), np.float32)
        for b in range(B):
            md[b, GQ * b:GQ * (b + 1)] = 1.0
        self.mdiag = md.astype(BF16)

        # per-chunk token validity [tot, 128]: g < ctx and g != pos
        g = ci[:, None] * CHUNK + np.arange(CHUNK)[None, :]
        s = self.chunk_seq[:, None]
        self.valid = ((g < ctx[s]) & (g != self.pos[s])).astype(np.float32)

        # flat gathered block list [tot*8]
        blk = []
        for b in range(B):
            blk.append(bt[b, :self.nch[b] * BPC])
        self.blocks_flat = np.concatenate(blk)


def _emit(nc, tile, mybir, sched):
    """Emit the per-core kernel (same NEFF for all cores)."""
    from concourse.masks import make_identity
    from concourse.tile import add_dep_helper

    dt = mybir.dt
    sc = sched
    TOT = sc.tot
    wsizes = _window_sizes(TOT)
    NWIN = len(wsizes)
    wstart = np.concatenate([[0], np.cumsum(wsizes)]).astype(int)

    # ---- DRAM I/O ----
    d_ht = nc.dram_tensor("ht", [128, 32, B], dt.bfloat16, kind="ExternalInput")
    d_wq = nc.dram_tensor("wq", [128, 32, 768], dt.bfloat16, kind="ExternalInput")
    d_wo = nc.dram_tensor("wo", [128, 4, HIDDEN], dt.bfloat16, kind="ExternalInput")
    d_bq = nc.dram_tensor("bq", [1, 768], dt.bfloat16, kind="ExternalInput")
    d_trig = nc.dram_tensor("trig", [B, 640], dt.float32, kind="ExternalInput")
    d_md = nc.dram_tensor("mdiag", [B, GQ * B], dt.bfloat16, kind="ExternalInput")
    d_kg = nc.dram_tensor("kg", [128, TOT * CHUNK], dt.float8e3, kind="ExternalInput")
    d_vh = nc.dram_tensor("vh", [128, TOT * VBF], dt.bfloat16, kind="ExternalInput")
    d_vl = nc.dram_tensor("vl", [128, TOT * VLW], dt.float8e3, kind="ExternalInput")
    d_out = nc.dram_tensor("out", [128, B * HIDDEN // 128], dt.float16,
                           kind="ExternalOutput")

    with tile.TileContext(nc) as tc:
        with (
            tc.tile_pool(name="const", bufs=1) as cp,
            tc.tile_pool(name="work", bufs=1) as wp,
            tc.tile_pool(name="kwp", bufs=3) as kwp,
            tc.tile_pool(name="vhp", bufs=2) as vhp,
            tc.tile_pool(name="vlp", bufs=2) as vlp,
            tc.tile_pool(name="vmp", bufs=3) as vmp,
            tc.tile_pool(name="extp", bufs=4) as extp,
            tc.tile_pool(name="pswork", bufs=1, space="PSUM") as pswork,
            tc.tile_pool(name="pssc", bufs=3, space="PSUM") as pssc,
            tc.tile_pool(name="psacc", bufs=1, space="PSUM") as psacc,
        ):
            # ---- critical-path DMAs first: ht then wq slices, alternating
            # between the two HWDGE rings so both drain the weights together.
            ht = cp.tile([128, 32, B], dt.bfloat16, tag="ht")
            nc.sync.dma_start(ht[:], d_ht[:])
            bq = cp.tile([1, 768], dt.bfloat16, tag="bq")
            nc.scalar.dma_start(bq[:], d_bq[:])
            trig = cp.tile([B, 640], dt.float32, tag="trig")
            nc.scalar.dma_start(trig[:], d_trig[:])
            mdiag = cp.tile([B, GQ * B], dt.bfloat16, tag="mdiag")
            nc.scalar.dma_start(mdiag[:], d_md[:])
            # wq: q-part (cols 0:512) first — it alone gates the attention
            # start; the k/v part (cols 512:768) is only needed by the
            # new-token path and rides the scalar ring later.
            wq = cp.tile([128, 32, 768], dt.bfloat16, tag="wq")
            wqq = []
            for qd in range(4):
                eng = nc.sync if qd % 2 == 0 else nc.scalar
                wqq.append(eng.dma_start(wq[:, 8 * qd:8 * (qd + 1), 0:512],
                                         d_wq[:, 8 * qd:8 * (qd + 1), 0:512]))
            cosf = trig[:, 0:320]
            sinf = trig[:, 320:640]
            wo = cp.tile([128, 4, HIDDEN], dt.bfloat16, tag="wo")

            zrow = cp.tile([128, 384], dt.bfloat16, tag="zrow")
            nc.vector.memset(zrow[:], 0.0)
            ones1 = cp.tile([1, 128], dt.bfloat16, tag="ones1")
            nc.vector.memset(ones1[:], 1.0)
            ones32 = cp.tile([32, 1], dt.bfloat16, tag="ones32")
            nc.vector.memset(ones32[:], 1.0)
            ident = cp.tile([128, 128], dt.bfloat16, tag="ident")
            make_identity(nc, ident[:])
            ebias = cp.tile([128, 1], dt.float32, tag="ebias")
            nc.vector.memset(ebias[:], EXP_BIAS)

            # ---- KV windows, all on the sync ring (its sequencer runs no
            # compute, so trigger instructions blocking on pool buffers are
            # harmless there). K is fp8; V comes as a bf16 hi-dim slice plus
            # an fp8 lo-dim+validity slice, merged on-chip by the DVE.
            kdmas = []
            wins = []
            for w in range(NWIN):
                c0 = int(wstart[w])
                c1 = int(wstart[w + 1])
                wsz = c1 - c0
                kwin = kwp.tile([128, CHUNK * WCH], dt.float8e3, tag="kw")
                kd = nc.sync.dma_start(kwin[:, :CHUNK * wsz],
                                       d_kg[:, CHUNK * c0:CHUNK * c1])
                kdmas.append(kd)
                vh = vhp.tile([128, VBF * WCH], dt.bfloat16, tag="vh")
                nc.sync.dma_start(vh[:, :VBF * wsz],
                                  d_vh[:, VBF * c0:VBF * c1])
                vl = vlp.tile([128, VLW * WCH], dt.float8e3, tag="vl")
                nc.sync.dma_start(vl[:, :VLW * wsz],
                                  d_vl[:, VLW * c0:VLW * c1])
                wins.append((kwin, vh, vl, c0, c1))

            # ---- PE warm-up fills (cover the pre-wq DMA latency only) ----
            wu = pssc.tile([32, 384], dt.float32, tag="sc")
            for i in range(12):
                nc.tensor.matmul(wu[:], lhsT=zrow[:, 0:32], rhs=zrow[:],
                                 start=True, stop=True, skip_group_check=True)

            # ---- QKV projection, q-part first: q[32, 512] = hT.T @ wq_q ----
            ps_qa = pswork.tile([B, 512], dt.float32, tag="mm")
            nc.tensor.matmul(ps_qa[:], lhsT=ones1[0:1, 0:B], rhs=bq[0:1, 0:512],
                             start=True, stop=False)
            for ki in range(32):
                nc.tensor.matmul(ps_qa[:], lhsT=ht[:, ki, :],
                                 rhs=wq[:, ki, 0:512], start=False, stop=ki == 31)
            qkv_f = wp.tile([B, 768], dt.float32, tag="qkvf")
            nc.vector.tensor_copy(qkv_f[:, 0:512], ps_qa[:])

            # ---- RoPE on the 4 q heads (rotate-half on the free axis) ----
            qk_rope = wp.tile([B, 768], dt.bfloat16, tag="qkrope")

            def rope(h0, h1):
                n = h1 - h0
                qs = qkv_f[:, 128 * h0:128 * h1].rearrange(
                    "p (h t x) -> p h t x", t=2, x=64)
                rs = qk_rope[:, 128 * h0:128 * h1].rearrange(
                    "p (h t x) -> p h t x", t=2, x=64)
                ch = c5[:, h0:h1, :]
                sh = s5[:, h0:h1, :]
                t1 = wp.tile([B, 5, 64], dt.float32, tag="t1")
                t2 = wp.tile([B, 5, 64], dt.float32, tag="t2")
                nc.vector.tensor_mul(t1[:, :n], qs[:, :, 0, :], ch)
                nc.vector.tensor_mul(t2[:, :n], qs[:, :, 1, :], sh)
                nc.vector.tensor_sub(rs[:, :, 0, :], t1[:, :n], t2[:, :n])
                t3 = wp.tile([B, 5, 64], dt.float32, tag="t1")
                t4 = wp.tile([B, 5, 64], dt.float32, tag="t2")
                nc.vector.tensor_mul(t3[:, :n], qs[:, :, 1, :], ch)
                nc.vector.tensor_mul(t4[:, :n], qs[:, :, 0, :], sh)
                nc.vector.tensor_add(rs[:, :, 1, :], t3[:, :n], t4[:, :n])

            c5 = cosf.rearrange("p (h x) -> p h x", x=64)
            s5 = sinf.rearrange("p (h x) -> p h x", x=64)
            rope(0, 4)

            # ---- transpose q heads: qt [128, 4b+h] ----
            qt = wp.tile([128, GQ * B], dt.bfloat16, tag="qt")
            kt_new = wp.tile([128, B], dt.bfloat16, tag="ktnew")
            for hh in range(4):
                pst = pswork.tile([128, B], dt.bfloat16, tag="tr", bufs=2)
                nc.tensor.transpose(
                    pst[:], qk_rope[:, 128 * hh:128 * (hh + 1)], ident[:B, :B]
                )
                nc.vector.tensor_copy(qt[:, hh::4], pst[:])

            vnew = wp.tile([B, 128], dt.bfloat16, tag="vnew")
            p2 = wp.tile([B, 128], dt.bfloat16, tag="p2")

            def emit_kv_path():
                # k/v part of the projection + new-token probabilities.
                # Emitted mid-wave-stream (in PE program order) so these
                # matmuls don't sit in the PE FIFO ahead of the first waves
                # while waiting on the late wq k/v slice.
                ps_qb = pswork.tile([B, 256], dt.float32, tag="qk1")
                nc.tensor.matmul(ps_qb[:], lhsT=ones1[0:1, 0:B],
                                 rhs=bq[0:1, 512:768], start=True, stop=False)
                for ki in range(32):
                    nc.tensor.matmul(ps_qb[:], lhsT=ht[:, ki, :],
                                     rhs=wq[:, ki, 512:768], start=False,
                                     stop=ki == 31)
                nc.vector.tensor_copy(qkv_f[:, 512:768], ps_qb[:])
                rope(4, 5)
                nc.vector.tensor_copy(vnew[:], qkv_f[:, 640:768])
                pst = pswork.tile([128, B], dt.bfloat16, tag="tr", bufs=2)
                nc.tensor.transpose(pst[:], qk_rope[:, 512:640], ident[:B, :B])
                nc.vector.tensor_copy(kt_new[:], pst[:])
                ps_x = pswork.tile([B, 128], dt.float32, tag="mm")
                nc.tensor.matmul(ps_x[:], lhsT=kt_new[:], rhs=qt[:],
                                 start=True, stop=True)
                extx = wp.tile([B, 128], dt.float32, tag="extx")
                nc.scalar.activation(
                    extx[:], ps_x[:], mybir.ActivationFunctionType.Exp,
                    bias=ebias[0:B, :], scale=SCALE,
                )
                nc.vector.tensor_mul(p2[:], extx[:], mdiag[:])

            # ---- zero the attention accumulator (data=0, defined has_written) ----
            ps_acc = psacc.tile([128, 256], dt.float32, tag="acc")
            nc.tensor.matmul(ps_acc[:, 0:256],
                             lhsT=zrow[:, 0:128], rhs=zrow[:, 0:256],
                             start=True, stop=False, skip_group_check=True)

            # ---- waves: scores pipelined two deep ahead of the AV pass ----
            waves = []
            for w, (kwin, vh, vl, c0, c1) in enumerate(wins):
                for ws in range(c0, c1, WAVE):
                    waves.append((w, kwin, c0, ws, min(WAVE, c1 - ws)))

            exts = {}
            vms = {}

            def emit_merge(w):
                # DVE merges the bf16 hi-dims and fp8 lo-dims+validity into
                # one bf16 V tile so the AV pass stays a 2-matmul structure.
                kwin, vh, vl, c0, c1 = wins[w]
                wsz = c1 - c0
                vm = vmp.tile([128, VW * WCH], dt.bfloat16, tag="vm",
                              name=f"vm{w}")
                vm3 = vm[:, :VW * wsz].rearrange("p (c x) -> p c x", x=VW)
                vh3 = vh[:, :VBF * wsz].rearrange("p (c x) -> p c x", x=VBF)
                vl3 = vl[:, :VLW * wsz].rearrange("p (c x) -> p c x", x=VLW)
                nc.vector.tensor_copy(vm3[:, :, 0:VBF], vh3)
                nc.vector.tensor_copy(vm3[:, :, VBF:VW], vl3)
                vms[w] = vm

            def emit_scores(i):
                w, kwin, c0, ws, n = waves[i]
                if w not in vms:
                    emit_merge(w)
                ps_sc = pssc.tile([128, 4 * WAVE], dt.float32, tag="sc",
                                  name=f"pssc{i}")
                for j in range(n):
                    ch = ws + j
                    l = ch - c0
                    b = int(sc.chunk_seq[ch])
                    nc.tensor.matmul(
                        ps_sc[:, 4 * j:4 * (j + 1)],
                        lhsT=kwin[:, CHUNK * l:CHUNK * (l + 1)],
                        rhs=qt[:, GQ * b:GQ * (b + 1)],
                        start=True, stop=True,
                    )
                ext = extp.tile([128, 4 * WAVE], dt.bfloat16, tag="ext",
                                name=f"ext{i}")
                nc.scalar.activation(
                    ext[:, :4 * n], ps_sc[:, :4 * n],
                    mybir.ActivationFunctionType.Exp,
                    bias=ebias[:], scale=SCALE,
                )
                exts[i] = ext

            emit_scores(0)
            emit_scores(1)
            for i in range(len(waves)):
                if i == 2:
                    emit_kv_path()
                if i + 2 < len(waves):
                    emit_scores(i + 2)  # PE runs two score-waves ahead of AV
                w, kwin, c0, ws, n = waves[i]
                vm = vms[w]
                ext = exts.pop(i)
                # uniform runs so the PE drain/fill overlap never breaks
                for j in range(n):
                    ch = ws + j
                    l = ch - c0
                    b = int(sc.chunk_seq[ch])
                    nc.tensor.matmul(
                        ps_acc[:, 4 * b:4 * (b + 1)],
                        lhsT=vm[:, VW * l:VW * l + 128],
                        rhs=ext[:, 4 * j:4 * (j + 1)],
                        start=False, stop=False, skip_group_check=True,
                    )
                for j in range(n):
                    ch = ws + j
                    l = ch - c0
                    b = int(sc.chunk_seq[ch])
                    nc.tensor.matmul(
                        ps_acc[0:1, 128 + 4 * b:132 + 4 * b],
                        lhsT=vm[:, VW * l + 128:VW * l + 129],
                        rhs=ext[:, 4 * j:4 * (j + 1)],
                        start=False, stop=False, skip_group_check=True,
                    )

            # ---- new-token contribution (precomputed p2 above) ----
            nc.tensor.matmul(ps_acc[:, 0:128], lhsT=vnew[:], rhs=p2[:],
                             start=False, stop=True, skip_group_check=True)
            nc.tensor.matmul(ps_acc[0:1, 128:256], lhsT=ones32[:], rhs=p2[:],
                             start=False, stop=True, skip_group_check=True)

            # ---- normalize: at = attn / denom ----
            # broadcast the sums across partitions FIRST, then a
            # 128-partition-parallel reciprocal (serial chain ~2x shorter)
            sums_bf = wp.tile([1, 128], dt.bfloat16, tag="sumsbf")
            nc.vector.tensor_copy(sums_bf[:], ps_acc[0:1, 128:256])
            ps_rb = pswork.tile([128, 128], dt.float32, tag="mm")
            nc.tensor.matmul(ps_rb[:], lhsT=ones1[0:1, :], rhs=sums_bf[0:1, :],
                             start=True, stop=True)
            rb_sb = wp.tile([128, 128], dt.float32, tag="rbsb")
            nc.vector.reciprocal(rb_sb[:], ps_rb[:])
            # head-major layout: at_hm[:, 32*h + s] = attn[:, 4*s + h]
            at_hm = wp.tile([128, 128], dt.bfloat16, tag="athm")
            nc.vector.tensor_mul(
                at_hm[:].rearrange("p (h s) -> p s h", h=4),
                ps_acc[:, 0:128].rearrange("p (s h) -> p s h", h=4),
                rb_sb[:].rearrange("p (s h) -> p s h", h=4),
            )

            # ---- O projection, transposed: outT[4096, 32] partial ----
            # lhsT = wo[:, h, 128j:128j+128] ([d, n] block), rhs = at_hm head
            # slice ([d, s]); psum groups of 4 j-blocks -> ostage [128, 1024]
            ostage = wp.tile([128, 32 * 32], dt.float16, tag="ostage")
            for g in range(8):
                ps_o = pswork.tile([128, 128], dt.float32, tag="tr", bufs=2)
                for jj in range(4):
                    j = 4 * g + jj
                    for hh in range(4):
                        nc.tensor.matmul(
                            ps_o[:, 32 * jj:32 * (jj + 1)],
                            lhsT=wo[:, hh, 128 * j:128 * (j + 1)],
                            rhs=at_hm[:, 32 * hh:32 * (hh + 1)],
                            start=(hh == 0),
                            stop=(hh == 3),
                            skip_group_check=True,
                        )
                if g % 2 == 0:
                    nc.scalar.copy(ostage[:, 128 * g:128 * (g + 1)], ps_o[:])
                else:
                    nc.vector.tensor_copy(ostage[:, 128 * g:128 * (g + 1)], ps_o[:])
                if g % 4 == 3:
                    nc.sync.dma_start(d_out[:, 128 * (g - 3):128 * (g + 1)],
                                      ostage[:, 128 * (g - 3):128 * (g + 1)])

    nc.compile()
    return nc


def _build_inputs(sched, hidden_states, W_qkv, b_qkv, W_o, k_cache, v_cache):
    """Per-core input maps with host-side gather into matmul-native layouts."""
    sc = sched
    TOT = sc.tot

    hts = hidden_states.T.astype(BF16)  # [4096, 32]
    ht_in = np.ascontiguousarray(hts.reshape(32, 128, B).transpose(1, 0, 2))

    # one global gather of the needed blocks (all kv heads at once)
    KB = k_cache[sc.blocks_flat]   # [TOT*8, 16, 8, 128] fp32
    VB = v_cache[sc.blocks_flat]

    maps = []
    for c in range(NCORES):
        qr = slice(512 * c, 512 * (c + 1))
        kr = slice(Q_SIZE + 128 * c, Q_SIZE + 128 * (c + 1))
        vr = slice(Q_SIZE + KV_SIZE + 128 * c, Q_SIZE + KV_SIZE + 128 * (c + 1))
        wq_sh = np.concatenate([W_qkv[qr], W_qkv[kr], W_qkv[vr]], axis=0)  # [768, 4096]
        wq_in = np.ascontiguousarray(
            wq_sh.T.astype(BF16).reshape(32, 128, 768).transpose(1, 0, 2))
        bq_sh = np.concatenate([b_qkv[qr], b_qkv[kr], b_qkv[vr]])
        bq_in = bq_sh[None, :].astype(BF16)
        wo_in = np.ascontiguousarray(
            W_o[:, qr].T.astype(BF16).reshape(4, 128, HIDDEN).transpose(1, 0, 2))

        # K: [TOT, 128 tok, 128 D] -> [128 D, TOT*128] fp8
        kc = KB[:, :, c, :].astype(FP8KV).reshape(TOT, CHUNK, D)
        kg_in = np.ascontiguousarray(
            kc.transpose(2, 0, 1).reshape(D, TOT * CHUNK))

        # V split: hi dims 0:VBF bf16, lo dims VBF:128 + validity fp8
        vc = VB[:, :, c, :].reshape(TOT, CHUNK, D) * sc.valid[:, :, None]
        vh_in = np.ascontiguousarray(
            vc[:, :, :VBF].astype(BF16).transpose(1, 0, 2).reshape(
                CHUNK, TOT * VBF))
        vlo = np.zeros((TOT, CHUNK, VLW), np.float32)
        vlo[:, :, :D - VBF] = vc[:, :, VBF:]
        vlo[:, :, D - VBF] = sc.valid
        vl_in = np.ascontiguousarray(
            vlo.astype(FP8KV).transpose(1, 0, 2).reshape(CHUNK, TOT * VLW))

        maps.append({
            "ht": ht_in, "wq": wq_in, "wo": wo_in, "bq": bq_in,
            "trig": np.concatenate([sc.cosf, sc.sinf], axis=1),
            "mdiag": sc.mdiag,
            "kg": kg_in, "vh": vh_in, "vl": vl_in,
        })
    return maps


_TRACE = {"on": False, "result": None}


def kernel(hidden_states, W_qkv, b_qkv, W_o, b_o, k_cache, v_cache,
           block_tables, context_lens):
    import concourse.tile as tile
    import concourse.mybir as mybir
    from concourse import bacc
    from concourse.bass_utils import run_bass_kernel_spmd

    sched = _Schedule(context_lens, block_tables)
    nc = bacc.Bacc("TRN2", target_bir_lowering=False, debug=False)
    _emit(nc, tile, mybir, sched)

    in_maps = _build_inputs(sched, np.asarray(hidden_states, np.float32),
                            np.asarray(W_qkv, np.float32),
                            np.asarray(b_qkv, np.float32),
                            np.asarray(W_o, np.float32),
                            np.asarray(k_cache, np.float32),
                            np.asarray(v_cache, np.float32))

    res = run_bass_kernel_spmd(nc, in_maps, core_ids=list(range(NCORES)),
                               trace=_TRACE["on"])
    _TRACE["result"] = res

    acc = np.zeros((B, HIDDEN), np.float64)
    for c in range(NCORES):
        o128 = res.results[c]["out"].astype(np.float64)  # [128, 1024]
        # o128[p, 32*j + s] = out[s, 128*j + p]
        acc += o128.reshape(128, 32, 32).transpose(2, 1, 0).reshape(B, HIDDEN)
    acc += np.asarray(b_o, np.float64)[None, :]
    return acc.astype(np.float32)


# revision 26
# speedup vs baseline: 1.1094x; 1.1053x over previous
"""Llama decode attention (paged KV, GQA) as a Bass/Tile kernel on 8 TRN2 cores.

Sharding: tensor-parallel by kv-head. Core c owns q heads 4c..4c+3, kv head c,
the matching W_qkv column shard, that kv-head's slice of the paged KV cache,
and the W_o row shard. Each core computes a partial [32, 4096] output; the
host sums the 8 partials (the "all-reduce") and adds b_o.

Host-side staging builds matmul-native KV layouts per core:
  - K: [128 (head dim), TOT*128 (chunk-major tokens)]  -> score matmul lhsT
  - V: [128 (token-in-chunk), TOT*129]; per chunk cols 0:128 = V rows,
    col 128 = validity (1.0 valid / 0.0 pad-or-new-token-slot), 129:132 pad.
    The validity column doubles as the softmax-denominator accumulator via a
    [tokens,1] x [tokens,4] matmul, so no masking ops are needed on device.
The new token's k/v (computed in-kernel from the QKV projection) enter
attention through one extra 32-token "chunk" (kt_new / vnew) with a
block-diagonal probability mask, so nothing is inserted into the KV tiles.

Schedule: the kernel is DMA-bound (~24 MB/core at ~370 GB/s). DMA order is
arranged so nothing steals bandwidth from the critical path:
  ht -> wq (8 slices, alternating HWDGE rings; QKV matmuls chase them
  ki-by-ki) -> K/V windows (K and V of each window on opposite rings to
  balance ring bytes) -> wo lands just before the last two windows ->
  fp16 output stores.
"""

import math

import numpy as np
import ml_dtypes

H = 32
KVH = 8
D = 128
HIDDEN = 4096
Q_SIZE = H * D
KV_SIZE = KVH * D
BLOCK = 16
NBLOCKS = 8192
MAXBPS = 128
MAXCTX = 2048
B = 32
NCORES = 8
GQ = H // NCORES          # q heads per core = 4
CHUNK = 128               # tokens per chunk
BPC = CHUNK // BLOCK      # blocks per chunk = 8
VW = 129                  # merged V chunk width: 128 D + 1 validity
VBF = 48                  # V head-dims kept in bf16 (rest fp8); K is all fp8
VLW = D - VBF + 1         # fp8 V slice width: 80 lo dims + validity = 81
WCH = 48                  # chunks per DMA window (max; tail windows taper)
WAVE = 16                 # chunks per exp wave
ROPE_THETA = 10000.0
SCALE = D ** -0.5
EXP_BIAS = -2.0           # exp(s*SCALE - 2): headroom vs overflow, cancels in norm

BF16 = ml_dtypes.bfloat16
FP16 = np.float16
FP8KV = ml_dtypes.float8_e3m4


def _ceil_div(a, b):
    return -(-a // b)


def _window_sizes(tot):
    """Full windows of WCH chunks; split the remainder so the final window is
    small (short post-stream PE tail)."""
    sizes = []
    rem = tot
    while rem > WCH:
        sizes.append(WCH)
        rem -= WCH
    if rem > 12:
        sizes.extend([rem - 12, 12])
    else:
        sizes.append(rem)
    return sizes


class _Schedule:
    """Static per-call schedule derived from context_lens/block_tables."""

    def __init__(self, context_lens, block_tables):
        ctx = np.asarray(context_lens, np.int64)
        bt = np.asarray(block_tables, np.int64)
        self.ctx = ctx
        self.bt = bt
        self.pos = ctx - 1
        self.nch = np.maximum(1, _ceil_div(ctx, CHUNK)).astype(np.int64)
        self.tot = int(self.nch.sum())
        self.chunk_seq = np.repeat(np.arange(B), self.nch)        # [tot]
        ci = np.concatenate([np.arange(n) for n in self.nch])
        self.chunk_ci = ci                                        # [tot]

        # RoPE tables at the new-token position
        half = D // 2
        inv_freq = 1.0 / (ROPE_THETA ** (np.arange(half, dtype=np.float64) / half))
        ang = self.pos[:, None].astype(np.float64) * inv_freq[None, :]
        self.cosf = np.tile(np.cos(ang).astype(np.float32), (1, 5))  # [32, 320]
        self.sinf = np.tile(np.sin(ang).astype(np.float32), (1, 5))

        # block-diagonal probability mask for the new-token chunk
        md = np.zeros((B, GQ * B), np.float32)
        for b in range(B):
            md[b, GQ * b:GQ * (b + 1)] = 1.0
        self.mdiag = md.astype(BF16)

        # per-chunk token validity [tot, 128]: g < ctx and g != pos
        g = ci[:, None] * CHUNK + np.arange(CHUNK)[None, :]
        s = self.chunk_seq[:, None]
        self.valid = ((g < ctx[s]) & (g != self.pos[s])).astype(np.float32)

        # flat gathered block list [tot*8]
        blk = []
        for b in range(B):
            blk.append(bt[b, :self.nch[b] * BPC])
        self.blocks_flat = np.concatenate(blk)


def _emit(nc, tile, mybir, sched):
    """Emit the per-core kernel (same NEFF for all cores)."""
    from concourse.masks import make_identity
    from concourse.tile import add_dep_helper

    dt = mybir.dt
    sc = sched
    TOT = sc.tot
    wsizes = _window_sizes(TOT)
    NWIN = len(wsizes)
    wstart = np.concatenate([[0], np.cumsum(wsizes)]).astype(int)

    # ---- DRAM I/O ----
    d_ht = nc.dram_tensor("ht", [128, 32, B], dt.bfloat16, kind="ExternalInput")
    d_wq = nc.dram_tensor("wq", [128, 32, 768], dt.bfloat16, kind="ExternalInput")
    d_wo = nc.dram_tensor("wo", [128, 4, HIDDEN], dt.bfloat16, kind="ExternalInput")
    d_bq = nc.dram_tensor("bq", [1, 768], dt.bfloat16, kind="ExternalInput")
    d_trig = nc.dram_tensor("trig", [B, 640], dt.float32, kind="ExternalInput")
    d_md = nc.dram_tensor("mdiag", [B, GQ * B], dt.bfloat16, kind="ExternalInput")
    d_kg = nc.dram_tensor("kg", [128, TOT * CHUNK], dt.float8e3, kind="ExternalInput")
    d_vh = nc.dram_tensor("vh", [128, TOT * VBF], dt.bfloat16, kind="ExternalInput")
    d_vl = nc.dram_tensor("vl", [128, TOT * VLW], dt.float8e3, kind="ExternalInput")
    d_out = nc.dram_tensor("out", [128, B * HIDDEN // 128], dt.float16,
                           kind="ExternalOutput")

    with tile.TileContext(nc) as tc:
        with (
            tc.tile_pool(name="const", bufs=1) as cp,
            tc.tile_pool(name="work", bufs=1) as wp,
            tc.tile_pool(name="kwp", bufs=3) as kwp,
            tc.tile_pool(name="vhp", bufs=2) as vhp,
            tc.tile_pool(name="vlp", bufs=2) as vlp,
            tc.tile_pool(name="vmp", bufs=3) as vmp,
            tc.tile_pool(name="extp", bufs=4) as extp,
            tc.tile_pool(name="pswork", bufs=1, space="PSUM") as pswork,
            tc.tile_pool(name="pssc", bufs=3, space="PSUM") as pssc,
            tc.tile_pool(name="psacc", bufs=1, space="PSUM") as psacc,
        ):
            # ---- critical-path DMAs first: ht then wq slices, alternating
            # between the two HWDGE rings so both drain the weights together.
            ht = cp.tile([128, 32, B], dt.bfloat16, tag="ht")
            nc.sync.dma_start(ht[:], d_ht[:])
            bq = cp.tile([1, 768], dt.bfloat16, tag="bq")
            nc.scalar.dma_start(bq[:], d_bq[:])
            trig = cp.tile([B, 640], dt.float32, tag="trig")
            nc.scalar.dma_start(trig[:], d_trig[:])
            mdiag = cp.tile([B, GQ * B], dt.bfloat16, tag="mdiag")
            nc.scalar.dma_start(mdiag[:], d_md[:])
            # wq split in 8 ki-slices alternating rings; the QKV matmuls
            # chase the slices ki-by-ki.
            wq = cp.tile([128, 32, 768], dt.bfloat16, tag="wq")
            for qd in range(8):
                eng = nc.sync if qd % 2 == 0 else nc.scalar
                eng.dma_start(wq[:, 4 * qd:4 * (qd + 1), :],
                              d_wq[:, 4 * qd:4 * (qd + 1), :])
            cosf = trig[:, 0:320]
            sinf = trig[:, 320:640]
            wo = cp.tile([128, 4, HIDDEN], dt.bfloat16, tag="wo")

            zrow = cp.tile([128, 384], dt.bfloat16, tag="zrow")
            nc.vector.memset(zrow[:], 0.0)
            ones1 = cp.tile([1, 128], dt.bfloat16, tag="ones1")
            nc.vector.memset(ones1[:], 1.0)
            ones32 = cp.tile([32, 1], dt.bfloat16, tag="ones32")
            nc.vector.memset(ones32[:], 1.0)
            ident = cp.tile([128, 128], dt.bfloat16, tag="ident")
            make_identity(nc, ident[:])
            ebias = cp.tile([128, 1], dt.float32, tag="ebias")
            nc.vector.memset(ebias[:], EXP_BIAS)

            # ---- KV windows, all on the sync ring (its sequencer runs no
            # compute, so trigger instructions blocking on pool buffers are
            # harmless there). K is fp8; V comes as a bf16 hi-dim slice plus
            # an fp8 lo-dim+validity slice, merged on-chip by the DVE.
            kdmas = []
            wins = []
            for w in range(NWIN):
                c0 = int(wstart[w])
                c1 = int(wstart[w + 1])
                wsz = c1 - c0
                kwin = kwp.tile([128, CHUNK * WCH], dt.float8e3, tag="kw")
                kd = nc.sync.dma_start(kwin[:, :CHUNK * wsz],
                                       d_kg[:, CHUNK * c0:CHUNK * c1])
                kdmas.append(kd)
                vh = vhp.tile([128, VBF * WCH], dt.bfloat16, tag="vh")
                nc.sync.dma_start(vh[:, :VBF * wsz],
                                  d_vh[:, VBF * c0:VBF * c1])
                vl = vlp.tile([128, VLW * WCH], dt.float8e3, tag="vl")
                nc.sync.dma_start(vl[:, :VLW * wsz],
                                  d_vl[:, VLW * c0:VLW * c1])
                wins.append((kwin, vh, vl, c0, c1))

            # wo on the (otherwise idle) gpsimd SWDGE ring; held until
            # window 1's K has landed so the prologue + first windows get
            # the full bandwidth.
            wo_dma = nc.gpsimd.dma_start(wo[:], d_wo[:])
            add_dep_helper(wo_dma.ins, kdmas[1].ins, sync=True,
                           reason="wo drains after window 1")

            # ---- PE warm-up fills (cover the pre-wq DMA latency only) ----
            wu = pssc.tile([32, 384], dt.float32, tag="sc")
            for i in range(12):
                nc.tensor.matmul(wu[:], lhsT=zrow[:, 0:32], rhs=zrow[:],
                                 start=True, stop=True, skip_group_check=True)

            # ---- QKV projection (single pass): qkv[32, 768] = hT.T @ wq ----
            ps_qa = pswork.tile([B, 512], dt.float32, tag="mm")
            ps_qb = pswork.tile([B, 256], dt.float32, tag="qk1")
            nc.tensor.matmul(ps_qa[:], lhsT=ones1[0:1, 0:B], rhs=bq[0:1, 0:512],
                             start=True, stop=False)
            nc.tensor.matmul(ps_qb[:], lhsT=ones1[0:1, 0:B], rhs=bq[0:1, 512:768],
                             start=True, stop=False)
            for ki in range(32):
                nc.tensor.matmul(ps_qa[:], lhsT=ht[:, ki, :],
                                 rhs=wq[:, ki, 0:512], start=False, stop=ki == 31)
                nc.tensor.matmul(ps_qb[:], lhsT=ht[:, ki, :],
                                 rhs=wq[:, ki, 512:768], start=False, stop=ki == 31)
            qkv_f = wp.tile([B, 768], dt.float32, tag="qkvf")
            nc.vector.tensor_copy(qkv_f[:, 0:512], ps_qa[:])
            nc.vector.tensor_copy(qkv_f[:, 512:768], ps_qb[:])

            # ---- RoPE on the 4 q heads (rotate-half on the free axis) ----
            qk_rope = wp.tile([B, 768], dt.bfloat16, tag="qkrope")

            def rope(h0, h1):
                n = h1 - h0
                qs = qkv_f[:, 128 * h0:128 * h1].rearrange(
                    "p (h t x) -> p h t x", t=2, x=64)
                rs = qk_rope[:, 128 * h0:128 * h1].rearrange(
                    "p (h t x) -> p h t x", t=2, x=64)
                ch = c5[:, h0:h1, :]
                sh = s5[:, h0:h1, :]
                t1 = wp.tile([B, 5, 64], dt.float32, tag="t1")
                t2 = wp.tile([B, 5, 64], dt.float32, tag="t2")
                nc.vector.tensor_mul(t1[:, :n], qs[:, :, 0, :], ch)
                nc.vector.tensor_mul(t2[:, :n], qs[:, :, 1, :], sh)
                nc.vector.tensor_sub(rs[:, :, 0, :], t1[:, :n], t2[:, :n])
                t3 = wp.tile([B, 5, 64], dt.float32, tag="t1")
                t4 = wp.tile([B, 5, 64], dt.float32, tag="t2")
                nc.vector.tensor_mul(t3[:, :n], qs[:, :, 1, :], ch)
                nc.vector.tensor_mul(t4[:, :n], qs[:, :, 0, :], sh)
                nc.vector.tensor_add(rs[:, :, 1, :], t3[:, :n], t4[:, :n])

            c5 = cosf.rearrange("p (h x) -> p h x", x=64)
            s5 = sinf.rearrange("p (h x) -> p h x", x=64)
            rope(0, 5)

            # ---- transpose q heads + k: qt [128, 4b+h], kt_new [128, 32] ----
            qt = wp.tile([128, GQ * B], dt.bfloat16, tag="qt")
            kt_new = wp.tile([128, B], dt.bfloat16, tag="ktnew")
            for hh in range(5):
                pst = pswork.tile([128, B], dt.bfloat16, tag="tr", bufs=2)
                nc.tensor.transpose(
                    pst[:], qk_rope[:, 128 * hh:128 * (hh + 1)], ident[:B, :B]
                )
                if hh < 4:
                    nc.vector.tensor_copy(qt[:, hh::4], pst[:])
                else:
                    nc.vector.tensor_copy(kt_new[:], pst[:])
            vnew = wp.tile([B, 128], dt.bfloat16, tag="vnew")
            nc.vector.tensor_copy(vnew[:], qkv_f[:, 640:768])

            # ---- new-token probabilities, hoisted off the critical tail ----
            ps_x = pswork.tile([B, 128], dt.float32, tag="mm")
            nc.tensor.matmul(ps_x[:], lhsT=kt_new[:], rhs=qt[:],
                             start=True, stop=True)
            extx = wp.tile([B, 128], dt.float32, tag="extx")
            nc.scalar.activation(
                extx[:], ps_x[:], mybir.ActivationFunctionType.Exp,
                bias=ebias[0:B, :], scale=SCALE,
            )
            p2 = wp.tile([B, 128], dt.bfloat16, tag="p2")
            nc.vector.tensor_mul(p2[:], extx[:], mdiag[:])

            # ---- zero the attention accumulator (data=0, defined has_written) ----
            ps_acc = psacc.tile([128, 256], dt.float32, tag="acc")
            nc.tensor.matmul(ps_acc[:, 0:256],
                             lhsT=zrow[:, 0:128], rhs=zrow[:, 0:256],
                             start=True, stop=False, skip_group_check=True)

            # ---- waves: scores pipelined two deep ahead of the AV pass ----
            waves = []
            for w, (kwin, vh, vl, c0, c1) in enumerate(wins):
                for ws in range(c0, c1, WAVE):
                    waves.append((w, kwin, c0, ws, min(WAVE, c1 - ws)))

            exts = {}
            vms = {}

            def emit_merge(w):
                # DVE merges the bf16 hi-dims and fp8 lo-dims+validity into
                # one bf16 V tile so the AV pass stays a 2-matmul structure.
                kwin, vh, vl, c0, c1 = wins[w]
                wsz = c1 - c0
                vm = vmp.tile([128, VW * WCH], dt.bfloat16, tag="vm",
                              name=f"vm{w}")
                vm3 = vm[:, :VW * wsz].rearrange("p (c x) -> p c x", x=VW)
                vh3 = vh[:, :VBF * wsz].rearrange("p (c x) -> p c x", x=VBF)
                vl3 = vl[:, :VLW * wsz].rearrange("p (c x) -> p c x", x=VLW)
                nc.vector.tensor_copy(vm3[:, :, 0:VBF], vh3)
                nc.vector.tensor_copy(vm3[:, :, VBF:VW], vl3)
                vms[w] = vm

            def emit_scores(i):
                w, kwin, c0, ws, n = waves[i]
                if w not in vms:
                    emit_merge(w)
                ps_sc = pssc.tile([128, 4 * WAVE], dt.float32, tag="sc",
                                  name=f"pssc{i}")
                for j in range(n):
                    ch = ws + j
                    l = ch - c0
                    b = int(sc.chunk_seq[ch])
                    nc.tensor.matmul(
                        ps_sc[:, 4 * j:4 * (j + 1)],
                        lhsT=kwin[:, CHUNK * l:CHUNK * (l + 1)],
                        rhs=qt[:, GQ * b:GQ * (b + 1)],
                        start=True, stop=True,
                    )
                ext = extp.tile([128, 4 * WAVE], dt.bfloat16, tag="ext",
                                name=f"ext{i}")
                nc.scalar.activation(
                    ext[:, :4 * n], ps_sc[:, :4 * n],
                    mybir.ActivationFunctionType.Exp,
                    bias=ebias[:], scale=SCALE,
                )
                exts[i] = ext

            emit_scores(0)
            emit_scores(1)
            for i in range(len(waves)):
                if i + 2 < len(waves):
                    emit_scores(i + 2)  # PE runs two score-waves ahead of AV
                w, kwin, c0, ws, n = waves[i]
                vm = vms[w]
                ext = exts.pop(i)
                # uniform runs so the PE drain/fill overlap never breaks
                for j in range(n):
                    ch = ws + j
                    l = ch - c0
                    b = int(sc.chunk_seq[ch])
                    nc.tensor.matmul(
                        ps_acc[:, 4 * b:4 * (b + 1)],
                        lhsT=vm[:, VW * l:VW * l + 128],
                        rhs=ext[:, 4 * j:4 * (j + 1)],
                        start=False, stop=False, skip_group_check=True,
                    )
                for j in range(n):
                    ch = ws + j
                    l = ch - c0
                    b = int(sc.chunk_seq[ch])
                    nc.tensor.matmul(
                        ps_acc[0:1, 128 + 4 * b:132 + 4 * b],
                        lhsT=vm[:, VW * l + 128:VW * l + 129],
                        rhs=ext[:, 4 * j:4 * (j + 1)],
                        start=False, stop=False, skip_group_check=True,
                    )

            # ---- new-token contribution (precomputed p2 above) ----
            nc.tensor.matmul(ps_acc[:, 0:128], lhsT=vnew[:], rhs=p2[:],
                             start=False, stop=True, skip_group_check=True)
            nc.tensor.matmul(ps_acc[0:1, 128:256], lhsT=ones32[:], rhs=p2[:],
                             start=False, stop=True, skip_group_check=True)

            # ---- normalize: at = attn / denom ----
            # broadcast the sums across partitions FIRST, then a
            # 128-partition-parallel reciprocal (serial chain ~2x shorter)
            sums_bf = wp.tile([1, 128], dt.bfloat16, tag="sumsbf")
            nc.vector.tensor_copy(sums_bf[:], ps_acc[0:1, 128:256])
            ps_rb = pswork.tile([128, 128], dt.float32, tag="mm")
            nc.tensor.matmul(ps_rb[:], lhsT=ones1[0:1, :], rhs=sums_bf[0:1, :],
                             start=True, stop=True)
            rb_sb = wp.tile([128, 128], dt.float32, tag="rbsb")
            nc.vector.reciprocal(rb_sb[:], ps_rb[:])
            # head-major layout: at_hm[:, 32*h + s] = attn[:, 4*s + h]
            at_hm = wp.tile([128, 128], dt.bfloat16, tag="athm")
            nc.vector.tensor_mul(
                at_hm[:].rearrange("p (h s) -> p s h", h=4),
                ps_acc[:, 0:128].rearrange("p (s h) -> p s h", h=4),
                rb_sb[:].rearrange("p (s h) -> p s h", h=4),
            )

            # ---- O projection, transposed: outT[4096, 32] partial ----
            # lhsT = wo[:, h, 128j:128j+128] ([d, n] block), rhs = at_hm head
            # slice ([d, s]); psum groups of 4 j-blocks -> ostage [128, 1024]
            ostage = wp.tile([128, 32 * 32], dt.float16, tag="ostage")
            for g in range(8):
                ps_o = pswork.tile([128, 128], dt.float32, tag="tr", bufs=2)
                for jj in range(4):
                    j = 4 * g + jj
                    for hh in range(4):
                        nc.tensor.matmul(
                            ps_o[:, 32 * jj:32 * (jj + 1)],
                            lhsT=wo[:, hh, 128 * j:128 * (j + 1)],
                            rhs=at_hm[:, 32 * hh:32 * (hh + 1)],
                            start=(hh == 0),
                            stop=(hh == 3),
                            skip_group_check=True,
                        )
                if g % 2 == 0:
                    nc.scalar.copy(ostage[:, 128 * g:128 * (g + 1)], ps_o[:])
                else:
                    nc.vector.tensor_copy(ostage[:, 128 * g:128 * (g + 1)], ps_o[:])
                if g % 4 == 3:
                    nc.sync.dma_start(d_out[:, 128 * (g - 3):128 * (g + 1)],
                                      ostage[:, 128 * (g - 3):128 * (g + 1)])

    nc.compile()
    return nc


def _build_inputs(sched, hidden_states, W_qkv, b_qkv, W_o, k_cache, v_cache):
    """Per-core input maps with host-side gather into matmul-native layouts."""
    sc = sched
    TOT = sc.tot

    hts = hidden_states.T.astype(BF16)  # [4096, 32]
    ht_in = np.ascontiguousarray(hts.reshape(32, 128, B).transpose(1, 0, 2))

    # one global gather of the needed blocks (all kv heads at once)
    KB = k_cache[sc.blocks_flat]   # [TOT*8, 16, 8, 128] fp32
    VB = v_cache[sc.blocks_flat]

    maps = []
    for c in range(NCORES):
        qr = slice(512 * c, 512 * (c + 1))
        kr = slice(Q_SIZE + 128 * c, Q_SIZE + 128 * (c + 1))
        vr = slice(Q_SIZE + KV_SIZE + 128 * c, Q_SIZE + KV_SIZE + 128 * (c + 1))
        wq_sh = np.concatenate([W_qkv[qr], W_qkv[kr], W_qkv[vr]], axis=0)  # [768, 4096]
        wq_in = np.ascontiguousarray(
            wq_sh.T.astype(BF16).reshape(32, 128, 768).transpose(1, 0, 2))
        bq_sh = np.concatenate([b_qkv[qr], b_qkv[kr], b_qkv[vr]])
        bq_in = bq_sh[None, :].astype(BF16)
        wo_in = np.ascontiguousarray(
            W_o[:, qr].T.astype(BF16).reshape(4, 128, HIDDEN).transpose(1, 0, 2))

        # K: [TOT, 128 tok, 128 D] -> [128 D, TOT*128] fp8
        kc = KB[:, :, c, :].astype(FP8KV).reshape(TOT, CHUNK, D)
        kg_in = np.ascontiguousarray(
            kc.transpose(2, 0, 1).reshape(D, TOT * CHUNK))

        # V split: hi dims 0:VBF bf16, lo dims VBF:128 + validity fp8
        vc = VB[:, :, c, :].reshape(TOT, CHUNK, D) * sc.valid[:, :, None]
        vh_in = np.ascontiguousarray(
            vc[:, :, :VBF].astype(BF16).transpose(1, 0, 2).reshape(
                CHUNK, TOT * VBF))
        vlo = np.zeros((TOT, CHUNK, VLW), np.float32)
        vlo[:, :, :D - VBF] = vc[:, :, VBF:]
        vlo[:, :, D - VBF] = sc.valid
        vl_in = np.ascontiguousarray(
            vlo.astype(FP8KV).transpose(1, 0, 2).reshape(CHUNK, TOT * VLW))

        maps.append({
            "ht": ht_in, "wq": wq_in, "wo": wo_in, "bq": bq_in,
            "trig": np.concatenate([sc.cosf, sc.sinf], axis=1),
            "mdiag": sc.mdiag,
            "kg": kg_in, "vh": vh_in, "vl": vl_in,
        })
    return maps


_TRACE = {"on": False, "result": None}


def kernel(hidden_states, W_qkv, b_qkv, W_o, b_o, k_cache, v_cache,
           block_tables, context_lens):
    import concourse.tile as tile
    import concourse.mybir as mybir
    from concourse import bacc
    from concourse.bass_utils import run_bass_kernel_spmd

    sched = _Schedule(context_lens, block_tables)
    nc = bacc.Bacc("TRN2", target_bir_lowering=False, debug=False)
    _emit(nc, tile, mybir, sched)

    in_maps = _build_inputs(sched, np.asarray(hidden_states, np.float32),
                            np.asarray(W_qkv, np.float32),
                            np.asarray(b_qkv, np.float32),
                            np.asarray(W_o, np.float32),
                            np.asarray(k_cache, np.float32),
                            np.asarray(v_cache, np.float32))

    res = run_bass_kernel_spmd(nc, in_maps, core_ids=list(range(NCORES)),
                               trace=_TRACE["on"])
    _TRACE["result"] = res

    acc = np.zeros((B, HIDDEN), np.float64)
    for c in range(NCORES):
        o128 = res.results[c]["out"].astype(np.float64)  # [128, 1024]
        # o128[p, 32*j + s] = out[s, 128*j + p]
        acc += o128.reshape(128, 32, 32).transpose(2, 1, 0).reshape(B, HIDDEN)
    acc += np.asarray(b_o, np.float64)[None, :]
    return acc.astype(np.float32)


# revision 27
# speedup vs baseline: 1.1511x; 1.0376x over previous
"""Llama decode attention (paged KV, GQA) as a Bass/Tile kernel on 8 TRN2 cores.

Sharding: tensor-parallel by kv-head. Core c owns q heads 4c..4c+3, kv head c,
the matching W_qkv column shard, that kv-head's slice of the paged KV cache,
and the W_o row shard. Each core computes a partial [32, 4096] output; the
host sums the 8 partials (the "all-reduce") and adds b_o.

Host-side staging builds matmul-native KV layouts per core:
  - K: [128 (head dim), TOT*128 (chunk-major tokens)]  -> score matmul lhsT
  - V: [128 (token-in-chunk), TOT*129]; per chunk cols 0:128 = V rows,
    col 128 = validity (1.0 valid / 0.0 pad-or-new-token-slot), 129:132 pad.
    The validity column doubles as the softmax-denominator accumulator via a
    [tokens,1] x [tokens,4] matmul, so no masking ops are needed on device.
The new token's k/v (computed in-kernel from the QKV projection) enter
attention through one extra 32-token "chunk" (kt_new / vnew) with a
block-diagonal probability mask, so nothing is inserted into the KV tiles.

Schedule: the kernel is DMA-bound (~24 MB/core at ~370 GB/s). DMA order is
arranged so nothing steals bandwidth from the critical path:
  ht -> wq (8 slices, alternating HWDGE rings; QKV matmuls chase them
  ki-by-ki) -> K/V windows (K and V of each window on opposite rings to
  balance ring bytes) -> wo lands just before the last two windows ->
  fp16 output stores.
"""

import math

import numpy as np
import ml_dtypes

H = 32
KVH = 8
D = 128
HIDDEN = 4096
Q_SIZE = H * D
KV_SIZE = KVH * D
BLOCK = 16
NBLOCKS = 8192
MAXBPS = 128
MAXCTX = 2048
B = 32
NCORES = 8
GQ = H // NCORES          # q heads per core = 4
CHUNK = 128               # tokens per chunk
BPC = CHUNK // BLOCK      # blocks per chunk = 8
VW = 129                  # merged V chunk width: 128 D + 1 validity
VBF = 48                  # V head-dims kept in bf16 (rest fp8); K is all fp8
VLW = D - VBF + 1         # fp8 V slice width: 80 lo dims + validity = 81
WCH = 48                  # chunks per DMA window (max; tail windows taper)
WAVE = 16                 # chunks per exp wave
ROPE_THETA = 10000.0
SCALE = D ** -0.5
EXP_BIAS = -2.0           # exp(s*SCALE - 2): headroom vs overflow, cancels in norm

BF16 = ml_dtypes.bfloat16
FP16 = np.float16
FP8KV = ml_dtypes.float8_e3m4


def _ceil_div(a, b):
    return -(-a // b)


def _window_sizes(tot):
    """Full windows of WCH chunks; split the remainder so the final window is
    small (short post-stream PE tail)."""
    sizes = []
    rem = tot
    while rem > WCH:
        sizes.append(WCH)
        rem -= WCH
    if rem > 12:
        sizes.extend([rem - 12, 12])
    else:
        sizes.append(rem)
    return sizes


class _Schedule:
    """Static per-call schedule derived from context_lens/block_tables."""

    def __init__(self, context_lens, block_tables):
        ctx = np.asarray(context_lens, np.int64)
        bt = np.asarray(block_tables, np.int64)
        self.ctx = ctx
        self.bt = bt
        self.pos = ctx - 1
        self.nch = np.maximum(1, _ceil_div(ctx, CHUNK)).astype(np.int64)
        self.tot = int(self.nch.sum())
        self.chunk_seq = np.repeat(np.arange(B), self.nch)        # [tot]
        ci = np.concatenate([np.arange(n) for n in self.nch])
        self.chunk_ci = ci                                        # [tot]

        # RoPE tables at the new-token position
        half = D // 2
        inv_freq = 1.0 / (ROPE_THETA ** (np.arange(half, dtype=np.float64) / half))
        ang = self.pos[:, None].astype(np.float64) * inv_freq[None, :]
        self.cosf = np.tile(np.cos(ang).astype(np.float32), (1, 5))  # [32, 320]
        self.sinf = np.tile(np.sin(ang).astype(np.float32), (1, 5))

        # block-diagonal probability mask for the new-token chunk
        md = np.zeros((B, GQ * B), np.float32)
        for b in range(B):
            md[b, GQ * b:GQ * (b + 1)] = 1.0
        self.mdiag = md.astype(BF16)

        # per-chunk token validity [tot, 128]: g < ctx and g != pos
        g = ci[:, None] * CHUNK + np.arange(CHUNK)[None, :]
        s = self.chunk_seq[:, None]
        self.valid = ((g < ctx[s]) & (g != self.pos[s])).astype(np.float32)

        # flat gathered block list [tot*8]
        blk = []
        for b in range(B):
            blk.append(bt[b, :self.nch[b] * BPC])
        self.blocks_flat = np.concatenate(blk)


def _emit(nc, tile, mybir, sched):
    """Emit the per-core kernel (same NEFF for all cores)."""
    from concourse.masks import make_identity
    from concourse.tile import add_dep_helper

    dt = mybir.dt
    sc = sched
    TOT = sc.tot
    wsizes = _window_sizes(TOT)
    NWIN = len(wsizes)
    wstart = np.concatenate([[0], np.cumsum(wsizes)]).astype(int)

    # ---- DRAM I/O ----
    d_ht = nc.dram_tensor("ht", [128, 32, B], dt.bfloat16, kind="ExternalInput")
    d_wq = nc.dram_tensor("wq", [128, 32, 768], dt.bfloat16, kind="ExternalInput")
    d_wo = nc.dram_tensor("wo", [128, 4, HIDDEN], dt.bfloat16, kind="ExternalInput")
    d_bq = nc.dram_tensor("bq", [1, 768], dt.bfloat16, kind="ExternalInput")
    d_trig = nc.dram_tensor("trig", [B, 640], dt.float32, kind="ExternalInput")
    d_md = nc.dram_tensor("mdiag", [B, GQ * B], dt.bfloat16, kind="ExternalInput")
    d_kg = nc.dram_tensor("kg", [128, TOT * CHUNK], dt.float8e3, kind="ExternalInput")
    d_vh = nc.dram_tensor("vh", [128, TOT * VBF], dt.bfloat16, kind="ExternalInput")
    d_vl = nc.dram_tensor("vl", [128, TOT * VLW], dt.float8e3, kind="ExternalInput")
    d_out = nc.dram_tensor("out", [128, B * HIDDEN // 128], dt.float16,
                           kind="ExternalOutput")

    with tile.TileContext(nc) as tc:
        with (
            tc.tile_pool(name="const", bufs=1) as cp,
            tc.tile_pool(name="work", bufs=1) as wp,
            tc.tile_pool(name="kwp", bufs=3) as kwp,
            tc.tile_pool(name="vhp", bufs=2) as vhp,
            tc.tile_pool(name="vlp", bufs=2) as vlp,
            tc.tile_pool(name="vmp", bufs=3) as vmp,
            tc.tile_pool(name="extp", bufs=4) as extp,
            tc.tile_pool(name="pswork", bufs=1, space="PSUM") as pswork,
            tc.tile_pool(name="pssc", bufs=3, space="PSUM") as pssc,
            tc.tile_pool(name="psacc", bufs=1, space="PSUM") as psacc,
        ):
            # ---- critical-path DMAs first: ht then wq slices, alternating
            # between the two HWDGE rings so both drain the weights together.
            ht = cp.tile([128, 32, B], dt.bfloat16, tag="ht")
            nc.sync.dma_start(ht[:], d_ht[:])
            bq = cp.tile([1, 768], dt.bfloat16, tag="bq")
            nc.scalar.dma_start(bq[:], d_bq[:])
            trig = cp.tile([B, 640], dt.float32, tag="trig")
            nc.scalar.dma_start(trig[:], d_trig[:])
            mdiag = cp.tile([B, GQ * B], dt.bfloat16, tag="mdiag")
            nc.scalar.dma_start(mdiag[:], d_md[:])
            # wq split in 8 ki-slices alternating rings; the QKV matmuls
            # chase the slices ki-by-ki.
            wq = cp.tile([128, 32, 768], dt.bfloat16, tag="wq")
            for qd in range(8):
                nc.sync.dma_start(wq[:, 4 * qd:4 * (qd + 1), :],
                                  d_wq[:, 4 * qd:4 * (qd + 1), :])
            cosf = trig[:, 0:320]
            sinf = trig[:, 320:640]
            wo = cp.tile([128, 4, HIDDEN], dt.bfloat16, tag="wo")

            zrow = cp.tile([128, 384], dt.bfloat16, tag="zrow")
            nc.vector.memset(zrow[:], 0.0)
            ones1 = cp.tile([1, 128], dt.bfloat16, tag="ones1")
            nc.vector.memset(ones1[:], 1.0)
            ones32 = cp.tile([32, 1], dt.bfloat16, tag="ones32")
            nc.vector.memset(ones32[:], 1.0)
            ident = cp.tile([128, 128], dt.bfloat16, tag="ident")
            make_identity(nc, ident[:])
            ebias = cp.tile([128, 1], dt.float32, tag="ebias")
            nc.vector.memset(ebias[:], EXP_BIAS)

            # ---- KV windows, all on the sync ring (its sequencer runs no
            # compute, so trigger instructions blocking on pool buffers are
            # harmless there). K is fp8; V comes as a bf16 hi-dim slice plus
            # an fp8 lo-dim+validity slice, merged on-chip by the DVE.
            kdmas = []
            wins = []
            for w in range(NWIN):
                c0 = int(wstart[w])
                c1 = int(wstart[w + 1])
                wsz = c1 - c0
                kwin = kwp.tile([128, CHUNK * WCH], dt.float8e3, tag="kw")
                kd = nc.sync.dma_start(kwin[:, :CHUNK * wsz],
                                       d_kg[:, CHUNK * c0:CHUNK * c1])
                kdmas.append(kd)
                vh = vhp.tile([128, VBF * WCH], dt.bfloat16, tag="vh")
                nc.sync.dma_start(vh[:, :VBF * wsz],
                                  d_vh[:, VBF * c0:VBF * c1])
                vl = vlp.tile([128, VLW * WCH], dt.float8e3, tag="vl")
                nc.sync.dma_start(vl[:, :VLW * wsz],
                                  d_vl[:, VLW * c0:VLW * c1])
                wins.append((kwin, vh, vl, c0, c1))

            # wo on the (otherwise idle) gpsimd SWDGE ring; held until
            # window 1's K has landed so the prologue + first windows get
            # the full bandwidth.
            wo_dma = nc.gpsimd.dma_start(wo[:], d_wo[:])
            add_dep_helper(wo_dma.ins, kdmas[1].ins, sync=True,
                           reason="wo drains after window 1")

            # ---- PE warm-up fills (cover the pre-wq DMA latency only) ----
            wu = pssc.tile([32, 384], dt.float32, tag="sc")
            for i in range(12):
                nc.tensor.matmul(wu[:], lhsT=zrow[:, 0:32], rhs=zrow[:],
                                 start=True, stop=True, skip_group_check=True)

            # ---- QKV projection (single pass): qkv[32, 768] = hT.T @ wq ----
            ps_qa = pswork.tile([B, 512], dt.float32, tag="mm")
            ps_qb = pswork.tile([B, 256], dt.float32, tag="qk1")
            nc.tensor.matmul(ps_qa[:], lhsT=ones1[0:1, 0:B], rhs=bq[0:1, 0:512],
                             start=True, stop=False)
            nc.tensor.matmul(ps_qb[:], lhsT=ones1[0:1, 0:B], rhs=bq[0:1, 512:768],
                             start=True, stop=False)
            for ki in range(32):
                nc.tensor.matmul(ps_qa[:], lhsT=ht[:, ki, :],
                                 rhs=wq[:, ki, 0:512], start=False, stop=ki == 31)
                nc.tensor.matmul(ps_qb[:], lhsT=ht[:, ki, :],
                                 rhs=wq[:, ki, 512:768], start=False, stop=ki == 31)
            qkv_f = wp.tile([B, 768], dt.float32, tag="qkvf")
            nc.scalar.copy(qkv_f[:, 0:512], ps_qa[:])
            nc.scalar.copy(qkv_f[:, 512:768], ps_qb[:])

            # ---- RoPE on the 4 q heads (rotate-half on the free axis) ----
            qk_rope = wp.tile([B, 768], dt.bfloat16, tag="qkrope")

            def rope(h0, h1):
                n = h1 - h0
                qs = qkv_f[:, 128 * h0:128 * h1].rearrange(
                    "p (h t x) -> p h t x", t=2, x=64)
                rs = qk_rope[:, 128 * h0:128 * h1].rearrange(
                    "p (h t x) -> p h t x", t=2, x=64)
                ch = c5[:, h0:h1, :]
                sh = s5[:, h0:h1, :]
                t1 = wp.tile([B, 5, 64], dt.float32, tag="t1")
                t2 = wp.tile([B, 5, 64], dt.float32, tag="t2")
                nc.vector.tensor_mul(t1[:, :n], qs[:, :, 0, :], ch)
                nc.vector.tensor_mul(t2[:, :n], qs[:, :, 1, :], sh)
                nc.vector.tensor_sub(rs[:, :, 0, :], t1[:, :n], t2[:, :n])
                t3 = wp.tile([B, 5, 64], dt.float32, tag="t1")
                t4 = wp.tile([B, 5, 64], dt.float32, tag="t2")
                nc.vector.tensor_mul(t3[:, :n], qs[:, :, 1, :], ch)
                nc.vector.tensor_mul(t4[:, :n], qs[:, :, 0, :], sh)
                nc.vector.tensor_add(rs[:, :, 1, :], t3[:, :n], t4[:, :n])

            c5 = cosf.rearrange("p (h x) -> p h x", x=64)
            s5 = sinf.rearrange("p (h x) -> p h x", x=64)
            rope(0, 5)

            # ---- transpose q heads + k: qt [128, 4b+h], kt_new [128, 32] ----
            qt = wp.tile([128, GQ * B], dt.bfloat16, tag="qt")
            kt_new = wp.tile([128, B], dt.bfloat16, tag="ktnew")
            for hh in range(5):
                pst = pswork.tile([128, B], dt.bfloat16, tag="tr", bufs=2)
                nc.tensor.transpose(
                    pst[:], qk_rope[:, 128 * hh:128 * (hh + 1)], ident[:B, :B]
                )
                if hh < 4:
                    nc.scalar.copy(qt[:, hh::4], pst[:])
                else:
                    nc.scalar.copy(kt_new[:], pst[:])
            vnew = wp.tile([B, 128], dt.bfloat16, tag="vnew")
            nc.vector.tensor_copy(vnew[:], qkv_f[:, 640:768])

            # ---- new-token probabilities, hoisted off the critical tail ----
            ps_x = pswork.tile([B, 128], dt.float32, tag="mm")
            nc.tensor.matmul(ps_x[:], lhsT=kt_new[:], rhs=qt[:],
                             start=True, stop=True)
            extx = wp.tile([B, 128], dt.float32, tag="extx")
            nc.scalar.activation(
                extx[:], ps_x[:], mybir.ActivationFunctionType.Exp,
                bias=ebias[0:B, :], scale=SCALE,
            )
            p2 = wp.tile([B, 128], dt.bfloat16, tag="p2")
            nc.vector.tensor_mul(p2[:], extx[:], mdiag[:])

            # ---- zero the attention accumulator (data=0, defined has_written) ----
            ps_acc = psacc.tile([128, 256], dt.float32, tag="acc")
            nc.tensor.matmul(ps_acc[:, 0:256],
                             lhsT=zrow[:, 0:128], rhs=zrow[:, 0:256],
                             start=True, stop=False, skip_group_check=True)

            # ---- waves: scores pipelined two deep ahead of the AV pass ----
            waves = []
            for w, (kwin, vh, vl, c0, c1) in enumerate(wins):
                for ws in range(c0, c1, WAVE):
                    waves.append((w, kwin, c0, ws, min(WAVE, c1 - ws)))

            exts = {}
            vms = {}

            def emit_merge(w):
                # DVE merges the bf16 hi-dims and fp8 lo-dims+validity into
                # one bf16 V tile so the AV pass stays a 2-matmul structure.
                kwin, vh, vl, c0, c1 = wins[w]
                wsz = c1 - c0
                vm = vmp.tile([128, VW * WCH], dt.bfloat16, tag="vm",
                              name=f"vm{w}")
                vm3 = vm[:, :VW * wsz].rearrange("p (c x) -> p c x", x=VW)
                vh3 = vh[:, :VBF * wsz].rearrange("p (c x) -> p c x", x=VBF)
                vl3 = vl[:, :VLW * wsz].rearrange("p (c x) -> p c x", x=VLW)
                nc.vector.tensor_copy(vm3[:, :, 0:VBF], vh3)
                nc.vector.tensor_copy(vm3[:, :, VBF:VW], vl3)
                vms[w] = vm

            def emit_scores(i):
                w, kwin, c0, ws, n = waves[i]
                if w not in vms:
                    emit_merge(w)
                ps_sc = pssc.tile([128, 4 * WAVE], dt.float32, tag="sc",
                                  name=f"pssc{i}")
                for j in range(n):
                    ch = ws + j
                    l = ch - c0
                    b = int(sc.chunk_seq[ch])
                    nc.tensor.matmul(
                        ps_sc[:, 4 * j:4 * (j + 1)],
                        lhsT=kwin[:, CHUNK * l:CHUNK * (l + 1)],
                        rhs=qt[:, GQ * b:GQ * (b + 1)],
                        start=True, stop=True,
                    )
                ext = extp.tile([128, 4 * WAVE], dt.bfloat16, tag="ext",
                                name=f"ext{i}")
                nc.scalar.activation(
                    ext[:, :4 * n], ps_sc[:, :4 * n],
                    mybir.ActivationFunctionType.Exp,
                    bias=ebias[:], scale=SCALE,
                )
                exts[i] = ext

            emit_scores(0)
            emit_scores(1)
            for i in range(len(waves)):
                if i + 2 < len(waves):
                    emit_scores(i + 2)  # PE runs two score-waves ahead of AV
                w, kwin, c0, ws, n = waves[i]
                vm = vms[w]
                ext = exts.pop(i)
                # uniform runs so the PE drain/fill overlap never breaks
                for j in range(n):
                    ch = ws + j
                    l = ch - c0
                    b = int(sc.chunk_seq[ch])
                    nc.tensor.matmul(
                        ps_acc[:, 4 * b:4 * (b + 1)],
                        lhsT=vm[:, VW * l:VW * l + 128],
                        rhs=ext[:, 4 * j:4 * (j + 1)],
                        start=False, stop=False, skip_group_check=True,
                    )
                for j in range(n):
                    ch = ws + j
                    l = ch - c0
                    b = int(sc.chunk_seq[ch])
                    nc.tensor.matmul(
                        ps_acc[0:1, 128 + 4 * b:132 + 4 * b],
                        lhsT=vm[:, VW * l + 128:VW * l + 129],
                        rhs=ext[:, 4 * j:4 * (j + 1)],
                        start=False, stop=False, skip_group_check=True,
                    )

            # ---- new-token contribution (precomputed p2 above) ----
            nc.tensor.matmul(ps_acc[:, 0:128], lhsT=vnew[:], rhs=p2[:],
                             start=False, stop=True, skip_group_check=True)
            nc.tensor.matmul(ps_acc[0:1, 128:256], lhsT=ones32[:], rhs=p2[:],
                             start=False, stop=True, skip_group_check=True)

            # ---- normalize: at = attn / denom ----
            # broadcast the sums across partitions FIRST, then a
            # 128-partition-parallel reciprocal (serial chain ~2x shorter)
            sums_bf = wp.tile([1, 128], dt.bfloat16, tag="sumsbf")
            nc.scalar.copy(sums_bf[:], ps_acc[0:1, 128:256])
            ps_rb = pswork.tile([128, 128], dt.float32, tag="mm")
            nc.tensor.matmul(ps_rb[:], lhsT=ones1[0:1, :], rhs=sums_bf[0:1, :],
                             start=True, stop=True)
            rb_sb = wp.tile([128, 128], dt.float32, tag="rbsb")
            nc.vector.reciprocal(rb_sb[:], ps_rb[:])
            # head-major layout: at_hm[:, 32*h + s] = attn[:, 4*s + h]
            at_hm = wp.tile([128, 128], dt.bfloat16, tag="athm")
            nc.vector.tensor_mul(
                at_hm[:].rearrange("p (h s) -> p s h", h=4),
                ps_acc[:, 0:128].rearrange("p (s h) -> p s h", h=4),
                rb_sb[:].rearrange("p (s h) -> p s h", h=4),
            )

            # ---- O projection, transposed: outT[4096, 32] partial ----
            # lhsT = wo[:, h, 128j:128j+128] ([d, n] block), rhs = at_hm head
            # slice ([d, s]); psum groups of 4 j-blocks -> ostage [128, 1024]
            ostage = wp.tile([128, 32 * 32], dt.float16, tag="ostage")
            for g in range(8):
                ps_o = pswork.tile([128, 128], dt.float32, tag="tr", bufs=2)
                for jj in range(4):
                    j = 4 * g + jj
                    for hh in range(4):
                        nc.tensor.matmul(
                            ps_o[:, 32 * jj:32 * (jj + 1)],
                            lhsT=wo[:, hh, 128 * j:128 * (j + 1)],
                            rhs=at_hm[:, 32 * hh:32 * (hh + 1)],
                            start=(hh == 0),
                            stop=(hh == 3),
                            skip_group_check=True,
                        )
                if g % 2 == 0:
                    nc.scalar.copy(ostage[:, 128 * g:128 * (g + 1)], ps_o[:])
                else:
                    nc.vector.tensor_copy(ostage[:, 128 * g:128 * (g + 1)], ps_o[:])
                if g % 4 == 3:
                    nc.sync.dma_start(d_out[:, 128 * (g - 3):128 * (g + 1)],
                                      ostage[:, 128 * (g - 3):128 * (g + 1)])

    nc.compile()
    return nc


def _build_inputs(sched, hidden_states, W_qkv, b_qkv, W_o, k_cache, v_cache):
    """Per-core input maps with host-side gather into matmul-native layouts."""
    sc = sched
    TOT = sc.tot

    hts = hidden_states.T.astype(BF16)  # [4096, 32]
    ht_in = np.ascontiguousarray(hts.reshape(32, 128, B).transpose(1, 0, 2))

    # one global gather of the needed blocks (all kv heads at once)
    KB = k_cache[sc.blocks_flat]   # [TOT*8, 16, 8, 128] fp32
    VB = v_cache[sc.blocks_flat]

    maps = []
    for c in range(NCORES):
        qr = slice(512 * c, 512 * (c + 1))
        kr = slice(Q_SIZE + 128 * c, Q_SIZE + 128 * (c + 1))
        vr = slice(Q_SIZE + KV_SIZE + 128 * c, Q_SIZE + KV_SIZE + 128 * (c + 1))
        wq_sh = np.concatenate([W_qkv[qr], W_qkv[kr], W_qkv[vr]], axis=0)  # [768, 4096]
        wq_in = np.ascontiguousarray(
            wq_sh.T.astype(BF16).reshape(32, 128, 768).transpose(1, 0, 2))
        bq_sh = np.concatenate([b_qkv[qr], b_qkv[kr], b_qkv[vr]])
        bq_in = bq_sh[None, :].astype(BF16)
        wo_in = np.ascontiguousarray(
            W_o[:, qr].T.astype(BF16).reshape(4, 128, HIDDEN).transpose(1, 0, 2))

        # K: [TOT, 128 tok, 128 D] -> [128 D, TOT*128] fp8
        kc = KB[:, :, c, :].astype(FP8KV).reshape(TOT, CHUNK, D)
        kg_in = np.ascontiguousarray(
            kc.transpose(2, 0, 1).reshape(D, TOT * CHUNK))

        # V split: hi dims 0:VBF bf16, lo dims VBF:128 + validity fp8
        vc = VB[:, :, c, :].reshape(TOT, CHUNK, D) * sc.valid[:, :, None]
        vh_in = np.ascontiguousarray(
            vc[:, :, :VBF].astype(BF16).transpose(1, 0, 2).reshape(
                CHUNK, TOT * VBF))
        vlo = np.zeros((TOT, CHUNK, VLW), np.float32)
        vlo[:, :, :D - VBF] = vc[:, :, VBF:]
        vlo[:, :, D - VBF] = sc.valid
        vl_in = np.ascontiguousarray(
            vlo.astype(FP8KV).transpose(1, 0, 2).reshape(CHUNK, TOT * VLW))

        maps.append({
            "ht": ht_in, "wq": wq_in, "wo": wo_in, "bq": bq_in,
            "trig": np.concatenate([sc.cosf, sc.sinf], axis=1),
            "mdiag": sc.mdiag,
            "kg": kg_in, "vh": vh_in, "vl": vl_in,
        })
    return maps


_TRACE = {"on": False, "result": None}


def kernel(hidden_states, W_qkv, b_qkv, W_o, b_o, k_cache, v_cache,
           block_tables, context_lens):
    import concourse.tile as tile
    import concourse.mybir as mybir
    from concourse import bacc
    from concourse.bass_utils import run_bass_kernel_spmd

    sched = _Schedule(context_lens, block_tables)
    nc = bacc.Bacc("TRN2", target_bir_lowering=False, debug=False)
    _emit(nc, tile, mybir, sched)

    in_maps = _build_inputs(sched, np.asarray(hidden_states, np.float32),
                            np.asarray(W_qkv, np.float32),
                            np.asarray(b_qkv, np.float32),
                            np.asarray(W_o, np.float32),
                            np.asarray(k_cache, np.float32),
                            np.asarray(v_cache, np.float32))

    res = run_bass_kernel_spmd(nc, in_maps, core_ids=list(range(NCORES)),
                               trace=_TRACE["on"])
    _TRACE["result"] = res

    acc = np.zeros((B, HIDDEN), np.float64)
    for c in range(NCORES):
        o128 = res.results[c]["out"].astype(np.float64)  # [128, 1024]
        # o128[p, 32*j + s] = out[s, 128*j + p]
        acc += o128.reshape(128, 32, 32).transpose(2, 1, 0).reshape(B, HIDDEN)
    acc += np.asarray(b_o, np.float64)[None, :]
    return acc.astype(np.float32)


# revision 31
# speedup vs baseline: 1.1549x; 1.0033x over previous
"""Llama decode attention (paged KV, GQA) as a Bass/Tile kernel on 8 TRN2 cores.

Sharding: tensor-parallel by kv-head. Core c owns q heads 4c..4c+3, kv head c,
the matching W_qkv column shard, that kv-head's slice of the paged KV cache,
and the W_o row shard. Each core computes a partial [32, 4096] output; the
host sums the 8 partials (the "all-reduce") and adds b_o.

Host-side staging builds matmul-native KV layouts per core:
  - K: [128 (head dim), TOT*128 (chunk-major tokens)] in fp8e3m4 -> score
    matmul lhsT (fp8 lhsT x bf16 rhs).
  - V arrives split by head dim to cut bytes while keeping the error under
    the gate: dims 0:48 in bf16 ("vh") and dims 48:128 plus a validity
    column in fp8e3m4 ("vl"). The DVE merges both into one bf16 tile
    [token, 129] per chunk, so the AV pass keeps its 2-matmul structure.
    The validity column doubles as the softmax-denominator accumulator via
    a [tokens,1] x [tokens,4] matmul, so no masking ops run on device.
The new token's k/v (computed in-kernel from the QKV projection) enter
attention through one extra 32-token "chunk" (kt_new / vnew) with a
block-diagonal probability mask, so nothing is inserted into the KV tiles.

Schedule: the kernel is DMA-bound (~21 MB/core at ~0.4 TB/s). Everything on
the critical path rides the sync HWDGE ring in strict FIFO order (a single
ring still spreads across all 16 SDMA engines): ht -> wq (8 ki-slices, the
QKV matmuls chase them) -> K/vh/vl windows (scores pipelined two waves
ahead of the AV pass). wo rides the gpsimd SWDGE ring, gated on window 1 so
it never steals prologue bandwidth. Small consts use the scalar ring; the
scalar engine's FIFO carries no window triggers, so exp waves and PSUM
copies run promptly. Output is stored as fp16 partials summed on host.
"""

import math

import numpy as np
import ml_dtypes

H = 32
KVH = 8
D = 128
HIDDEN = 4096
Q_SIZE = H * D
KV_SIZE = KVH * D
BLOCK = 16
NBLOCKS = 8192
MAXBPS = 128
MAXCTX = 2048
B = 32
NCORES = 8
GQ = H // NCORES          # q heads per core = 4
CHUNK = 128               # tokens per chunk
BPC = CHUNK // BLOCK      # blocks per chunk = 8
VW = 129                  # merged V chunk width: 128 D + 1 validity
VBF = 48                  # V head-dims kept in bf16 (rest fp8); K is all fp8
VLW = D - VBF + 1         # fp8 V slice width: 80 lo dims + validity = 81
WCH = 48                  # chunks per DMA window (max; tail windows taper)
WAVE = 16                 # chunks per exp wave
ROPE_THETA = 10000.0
SCALE = D ** -0.5
EXP_BIAS = -2.0           # exp(s*SCALE - 2): headroom vs overflow, cancels in norm

BF16 = ml_dtypes.bfloat16
FP16 = np.float16
FP8KV = ml_dtypes.float8_e3m4


def _ceil_div(a, b):
    return -(-a // b)


def _window_sizes(tot):
    """Full windows of WCH chunks; split the remainder so the final window is
    small (short post-stream PE tail)."""
    sizes = []
    rem = tot
    while rem > WCH:
        sizes.append(WCH)
        rem -= WCH
    if rem > 12:
        sizes.extend([rem - 12, 12])
    else:
        sizes.append(rem)
    return sizes


class _Schedule:
    """Static per-call schedule derived from context_lens/block_tables."""

    def __init__(self, context_lens, block_tables):
        ctx = np.asarray(context_lens, np.int64)
        bt = np.asarray(block_tables, np.int64)
        self.ctx = ctx
        self.bt = bt
        self.pos = ctx - 1
        self.nch = np.maximum(1, _ceil_div(ctx, CHUNK)).astype(np.int64)
        self.tot = int(self.nch.sum())
        self.chunk_seq = np.repeat(np.arange(B), self.nch)        # [tot]
        ci = np.concatenate([np.arange(n) for n in self.nch])
        self.chunk_ci = ci                                        # [tot]

        # RoPE tables at the new-token position
        half = D // 2
        inv_freq = 1.0 / (ROPE_THETA ** (np.arange(half, dtype=np.float64) / half))
        ang = self.pos[:, None].astype(np.float64) * inv_freq[None, :]
        self.cosf = np.tile(np.cos(ang).astype(np.float32), (1, 10))  # [32, 640]
        self.sinf = np.tile(np.sin(ang).astype(np.float32), (1, 10))

        # block-diagonal probability mask for the new-token chunk
        md = np.zeros((B, GQ * B), np.float32)
        for b in range(B):
            md[b, GQ * b:GQ * (b + 1)] = 1.0
        self.mdiag = md.astype(BF16)

        # per-chunk token validity [tot, 128]: g < ctx and g != pos
        g = ci[:, None] * CHUNK + np.arange(CHUNK)[None, :]
        s = self.chunk_seq[:, None]
        self.valid = ((g < ctx[s]) & (g != self.pos[s])).astype(np.float32)

        # flat gathered block list [tot*8]
        blk = []
        for b in range(B):
            blk.append(bt[b, :self.nch[b] * BPC])
        self.blocks_flat = np.concatenate(blk)


def _emit(nc, tile, mybir, sched):
    """Emit the per-core kernel (same NEFF for all cores)."""
    from concourse.masks import make_identity
    from concourse.tile import add_dep_helper

    dt = mybir.dt
    sc = sched
    TOT = sc.tot
    wsizes = _window_sizes(TOT)
    NWIN = len(wsizes)
    wstart = np.concatenate([[0], np.cumsum(wsizes)]).astype(int)

    # ---- DRAM I/O ----
    d_ht = nc.dram_tensor("ht", [128, 32, B], dt.bfloat16, kind="ExternalInput")
    d_wq = nc.dram_tensor("wq", [128, 32, 768], dt.bfloat16, kind="ExternalInput")
    d_wo = nc.dram_tensor("wo", [128, 4, HIDDEN], dt.bfloat16, kind="ExternalInput")
    d_bq = nc.dram_tensor("bq", [1, 768], dt.bfloat16, kind="ExternalInput")
    d_trig = nc.dram_tensor("trig", [B, 1280], dt.float32, kind="ExternalInput")
    d_md = nc.dram_tensor("mdiag", [B, GQ * B], dt.bfloat16, kind="ExternalInput")
    d_kg = nc.dram_tensor("kg", [128, TOT * CHUNK], dt.float8e3, kind="ExternalInput")
    d_vh = nc.dram_tensor("vh", [128, TOT * VBF], dt.bfloat16, kind="ExternalInput")
    d_vl = nc.dram_tensor("vl", [128, TOT * VLW], dt.float8e3, kind="ExternalInput")
    d_out = nc.dram_tensor("out", [128, B * HIDDEN // 128], dt.float16,
                           kind="ExternalOutput")

    with tile.TileContext(nc) as tc:
        with (
            tc.tile_pool(name="const", bufs=1) as cp,
            tc.tile_pool(name="work", bufs=1) as wp,
            tc.tile_pool(name="kwp", bufs=3) as kwp,
            tc.tile_pool(name="vhp", bufs=2) as vhp,
            tc.tile_pool(name="vlp", bufs=2) as vlp,
            tc.tile_pool(name="vmp", bufs=3) as vmp,
            tc.tile_pool(name="extp", bufs=4) as extp,
            tc.tile_pool(name="pswork", bufs=1, space="PSUM") as pswork,
            tc.tile_pool(name="pssc", bufs=3, space="PSUM") as pssc,
            tc.tile_pool(name="psacc", bufs=1, space="PSUM") as psacc,
        ):
            # ---- critical-path DMAs first: ht then wq slices, alternating
            # between the two HWDGE rings so both drain the weights together.
            ht = cp.tile([128, 32, B], dt.bfloat16, tag="ht")
            nc.sync.dma_start(ht[:], d_ht[:])
            bq = cp.tile([1, 768], dt.bfloat16, tag="bq")
            nc.scalar.dma_start(bq[:], d_bq[:])
            trig = cp.tile([B, 1280], dt.float32, tag="trig")
            nc.scalar.dma_start(trig[:], d_trig[:])
            mdiag = cp.tile([B, GQ * B], dt.bfloat16, tag="mdiag")
            nc.scalar.dma_start(mdiag[:], d_md[:])
            # wq split in 8 ki-slices alternating rings; the QKV matmuls
            # chase the slices ki-by-ki.
            wq = cp.tile([128, 32, 768], dt.bfloat16, tag="wq")
            for qd in range(8):
                nc.sync.dma_start(wq[:, 4 * qd:4 * (qd + 1), :],
                                  d_wq[:, 4 * qd:4 * (qd + 1), :])
            cosf = trig[:, 0:640]
            sinf = trig[:, 640:1280]
            wo = cp.tile([128, 4, HIDDEN], dt.bfloat16, tag="wo")

            zrow = cp.tile([128, 384], dt.bfloat16, tag="zrow")
            nc.vector.memset(zrow[:], 0.0)
            ones1 = cp.tile([1, 128], dt.bfloat16, tag="ones1")
            nc.vector.memset(ones1[:], 1.0)
            ones32 = cp.tile([32, 1], dt.bfloat16, tag="ones32")
            nc.vector.memset(ones32[:], 1.0)
            ident = cp.tile([128, 128], dt.bfloat16, tag="ident")
            make_identity(nc, ident[:])
            ebias = cp.tile([128, 1], dt.float32, tag="ebias")
            nc.vector.memset(ebias[:], EXP_BIAS)

            # ---- KV windows, all on the sync ring (its sequencer runs no
            # compute, so trigger instructions blocking on pool buffers are
            # harmless there). K is fp8; V comes as a bf16 hi-dim slice plus
            # an fp8 lo-dim+validity slice, merged on-chip by the DVE.
            kdmas = []
            wins = []
            for w in range(NWIN):
                c0 = int(wstart[w])
                c1 = int(wstart[w + 1])
                wsz = c1 - c0
                kwin = kwp.tile([128, CHUNK * WCH], dt.float8e3, tag="kw")
                kd = nc.sync.dma_start(kwin[:, :CHUNK * wsz],
                                       d_kg[:, CHUNK * c0:CHUNK * c1])
                kdmas.append(kd)
                vh = vhp.tile([128, VBF * WCH], dt.bfloat16, tag="vh")
                nc.sync.dma_start(vh[:, :VBF * wsz],
                                  d_vh[:, VBF * c0:VBF * c1])
                vl = vlp.tile([128, VLW * WCH], dt.float8e3, tag="vl")
                nc.sync.dma_start(vl[:, :VLW * wsz],
                                  d_vl[:, VLW * c0:VLW * c1])
                wins.append((kwin, vh, vl, c0, c1))

            # wo on the (otherwise idle) gpsimd SWDGE ring; held until
            # window 1's K has landed so the prologue + first windows get
            # the full bandwidth.
            wo_dma = nc.gpsimd.dma_start(wo[:], d_wo[:])
            add_dep_helper(wo_dma.ins, kdmas[2].ins, sync=True,
                           reason="wo drains after window 1")

            # ---- PE warm-up fills (cover the pre-wq DMA latency only) ----
            wu = pssc.tile([32, 384], dt.float32, tag="sc")
            for i in range(12):
                nc.tensor.matmul(wu[:], lhsT=zrow[:, 0:32], rhs=zrow[:],
                                 start=True, stop=True, skip_group_check=True)

            # ---- QKV projection (single pass): qkv[32, 768] = hT.T @ wq ----
            ps_qa = pswork.tile([B, 512], dt.float32, tag="mm")
            ps_qb = pswork.tile([B, 256], dt.float32, tag="qk1")
            nc.tensor.matmul(ps_qa[:], lhsT=ones1[0:1, 0:B], rhs=bq[0:1, 0:512],
                             start=True, stop=False)
            nc.tensor.matmul(ps_qb[:], lhsT=ones1[0:1, 0:B], rhs=bq[0:1, 512:768],
                             start=True, stop=False)
            for ki in range(32):
                nc.tensor.matmul(ps_qa[:], lhsT=ht[:, ki, :],
                                 rhs=wq[:, ki, 0:512], start=False, stop=ki == 31)
                nc.tensor.matmul(ps_qb[:], lhsT=ht[:, ki, :],
                                 rhs=wq[:, ki, 512:768], start=False, stop=ki == 31)
            qkv_f = wp.tile([B, 768], dt.float32, tag="qkvf")
            nc.scalar.copy(qkv_f[:, 0:512], ps_qa[:])
            nc.scalar.copy(qkv_f[:, 512:768], ps_qb[:])

            # ---- RoPE on the 4 q heads (rotate-half on the free axis) ----
            qk_rope = wp.tile([B, 768], dt.bfloat16, tag="qkrope")

            # RoPE in 4 wide DVE ops (doubled host trig tables): fewer
            # per-op overheads than the 6-narrow-op rotate-half form.
            tcos = wp.tile([B, 640], dt.float32, tag="t1")
            tsin = wp.tile([B, 640], dt.float32, tag="t2")
            nc.vector.tensor_mul(tcos[:], qkv_f[:, 0:640], cosf)
            nc.vector.tensor_mul(tsin[:], qkv_f[:, 0:640], sinf)
            tc4 = tcos[:].rearrange("p (h t x) -> p h t x", t=2, x=64)
            ts4 = tsin[:].rearrange("p (h t x) -> p h t x", t=2, x=64)
            rr4 = qk_rope[:, 0:640].rearrange("p (h t x) -> p h t x", t=2, x=64)
            nc.vector.tensor_sub(rr4[:, :, 0, :], tc4[:, :, 0, :], ts4[:, :, 1, :])
            nc.vector.tensor_add(rr4[:, :, 1, :], tc4[:, :, 1, :], ts4[:, :, 0, :])

            # ---- transpose q heads + k: qt [128, 4b+h], kt_new [128, 32] ----
            qt = wp.tile([128, GQ * B], dt.bfloat16, tag="qt")
            kt_new = wp.tile([128, B], dt.bfloat16, tag="ktnew")
            for hh in range(5):
                pst = pswork.tile([128, B], dt.bfloat16, tag="tr", bufs=2)
                nc.tensor.transpose(
                    pst[:], qk_rope[:, 128 * hh:128 * (hh + 1)], ident[:B, :B]
                )
                if hh < 4:
                    nc.scalar.copy(qt[:, hh::4], pst[:])
                else:
                    nc.scalar.copy(kt_new[:], pst[:])
            vnew = wp.tile([B, 128], dt.bfloat16, tag="vnew")
            nc.vector.tensor_copy(vnew[:], qkv_f[:, 640:768])

            # ---- new-token probabilities, hoisted off the critical tail ----
            ps_x = pswork.tile([B, 128], dt.float32, tag="mm")
            nc.tensor.matmul(ps_x[:], lhsT=kt_new[:], rhs=qt[:],
                             start=True, stop=True)
            extx = wp.tile([B, 128], dt.float32, tag="extx")
            nc.scalar.activation(
                extx[:], ps_x[:], mybir.ActivationFunctionType.Exp,
                bias=ebias[0:B, :], scale=SCALE,
            )
            p2 = wp.tile([B, 128], dt.bfloat16, tag="p2")
            nc.vector.tensor_mul(p2[:], extx[:], mdiag[:])

            # ---- zero the attention accumulator (data=0, defined has_written) ----
            ps_acc = psacc.tile([128, 256], dt.float32, tag="acc")
            nc.tensor.matmul(ps_acc[:, 0:256],
                             lhsT=zrow[:, 0:128], rhs=zrow[:, 0:256],
                             start=True, stop=False, skip_group_check=True)

            # ---- waves: scores pipelined two deep ahead of the AV pass ----
            waves = []
            for w, (kwin, vh, vl, c0, c1) in enumerate(wins):
                for ws in range(c0, c1, WAVE):
                    waves.append((w, kwin, c0, ws, min(WAVE, c1 - ws)))

            exts = {}
            vms = {}

            def emit_merge(w):
                # DVE merges the bf16 hi-dims and fp8 lo-dims+validity into
                # one bf16 V tile so the AV pass stays a 2-matmul structure.
                kwin, vh, vl, c0, c1 = wins[w]
                wsz = c1 - c0
                vm = vmp.tile([128, VW * WCH], dt.bfloat16, tag="vm",
                              name=f"vm{w}")
                vm3 = vm[:, :VW * wsz].rearrange("p (c x) -> p c x", x=VW)
                vh3 = vh[:, :VBF * wsz].rearrange("p (c x) -> p c x", x=VBF)
                vl3 = vl[:, :VLW * wsz].rearrange("p (c x) -> p c x", x=VLW)
                nc.vector.tensor_copy(vm3[:, :, 0:VBF], vh3)
                nc.vector.tensor_copy(vm3[:, :, VBF:VW], vl3)
                vms[w] = vm

            def emit_scores(i):
                w, kwin, c0, ws, n = waves[i]
                if w not in vms:
                    emit_merge(w)
                ps_sc = pssc.tile([128, 4 * WAVE], dt.float32, tag="sc",
                                  name=f"pssc{i}")
                for j in range(n):
                    ch = ws + j
                    l = ch - c0
                    b = int(sc.chunk_seq[ch])
                    nc.tensor.matmul(
                        ps_sc[:, 4 * j:4 * (j + 1)],
                        lhsT=kwin[:, CHUNK * l:CHUNK * (l + 1)],
                        rhs=qt[:, GQ * b:GQ * (b + 1)],
                        start=True, stop=True,
                    )
                ext = extp.tile([128, 4 * WAVE], dt.bfloat16, tag="ext",
                                name=f"ext{i}")
                nc.scalar.activation(
                    ext[:, :4 * n], ps_sc[:, :4 * n],
                    mybir.ActivationFunctionType.Exp,
                    bias=ebias[:], scale=SCALE,
                )
                exts[i] = ext

            emit_scores(0)
            emit_scores(1)
            for i in range(len(waves)):
                if i + 2 < len(waves):
                    emit_scores(i + 2)  # PE runs two score-waves ahead of AV
                w, kwin, c0, ws, n = waves[i]
                vm = vms[w]
                ext = exts.pop(i)
                # uniform runs so the PE drain/fill overlap never breaks
                for j in range(n):
                    ch = ws + j
                    l = ch - c0
                    b = int(sc.chunk_seq[ch])
                    nc.tensor.matmul(
                        ps_acc[:, 4 * b:4 * (b + 1)],
                        lhsT=vm[:, VW * l:VW * l + 128],
                        rhs=ext[:, 4 * j:4 * (j + 1)],
                        start=False, stop=False, skip_group_check=True,
                    )
                for j in range(n):
                    ch = ws + j
                    l = ch - c0
                    b = int(sc.chunk_seq[ch])
                    nc.tensor.matmul(
                        ps_acc[0:1, 128 + 4 * b:132 + 4 * b],
                        lhsT=vm[:, VW * l + 128:VW * l + 129],
                        rhs=ext[:, 4 * j:4 * (j + 1)],
                        start=False, stop=False, skip_group_check=True,
                    )

            # ---- new-token contribution (precomputed p2 above) ----
            nc.tensor.matmul(ps_acc[:, 0:128], lhsT=vnew[:], rhs=p2[:],
                             start=False, stop=True, skip_group_check=True)
            nc.tensor.matmul(ps_acc[0:1, 128:256], lhsT=ones32[:], rhs=p2[:],
                             start=False, stop=True, skip_group_check=True)

            # ---- normalize: at = attn / denom ----
            # broadcast the sums across partitions FIRST, then a
            # 128-partition-parallel reciprocal (serial chain ~2x shorter)
            sums_bf = wp.tile([1, 128], dt.bfloat16, tag="sumsbf")
            nc.scalar.copy(sums_bf[:], ps_acc[0:1, 128:256])
            ps_rb = pswork.tile([128, 128], dt.float32, tag="mm")
            nc.tensor.matmul(ps_rb[:], lhsT=ones1[0:1, :], rhs=sums_bf[0:1, :],
                             start=True, stop=True)
            rb_sb = wp.tile([128, 128], dt.float32, tag="rbsb")
            nc.vector.reciprocal(rb_sb[:], ps_rb[:])
            # head-major layout: at_hm[:, 32*h + s] = attn[:, 4*s + h]
            at_hm = wp.tile([128, 128], dt.bfloat16, tag="athm")
            nc.vector.tensor_mul(
                at_hm[:].rearrange("p (h s) -> p s h", h=4),
                ps_acc[:, 0:128].rearrange("p (s h) -> p s h", h=4),
                rb_sb[:].rearrange("p (s h) -> p s h", h=4),
            )

            # ---- O projection, transposed: outT[4096, 32] partial ----
            # lhsT = wo[:, h, 128j:128j+128] ([d, n] block), rhs = at_hm head
            # slice ([d, s]); psum groups of 4 j-blocks -> ostage [128, 1024]
            ostage = wp.tile([128, 32 * 32], dt.float16, tag="ostage")
            for g in range(8):
                ps_o = pswork.tile([128, 128], dt.float32, tag="tr", bufs=2)
                for jj in range(4):
                    j = 4 * g + jj
                    for hh in range(4):
                        nc.tensor.matmul(
                            ps_o[:, 32 * jj:32 * (jj + 1)],
                            lhsT=wo[:, hh, 128 * j:128 * (j + 1)],
                            rhs=at_hm[:, 32 * hh:32 * (hh + 1)],
                            start=(hh == 0),
                            stop=(hh == 3),
                            skip_group_check=True,
                        )
                if g % 2 == 0:
                    nc.scalar.copy(ostage[:, 128 * g:128 * (g + 1)], ps_o[:])
                else:
                    nc.vector.tensor_copy(ostage[:, 128 * g:128 * (g + 1)], ps_o[:])
                if g % 4 == 3:
                    nc.sync.dma_start(d_out[:, 128 * (g - 3):128 * (g + 1)],
                                      ostage[:, 128 * (g - 3):128 * (g + 1)])

    nc.compile()
    return nc


def _build_inputs(sched, hidden_states, W_qkv, b_qkv, W_o, k_cache, v_cache):
    """Per-core input maps with host-side gather into matmul-native layouts."""
    sc = sched
    TOT = sc.tot

    hts = hidden_states.T.astype(BF16)  # [4096, 32]
    ht_in = np.ascontiguousarray(hts.reshape(32, 128, B).transpose(1, 0, 2))

    # one global gather of the needed blocks (all kv heads at once)
    KB = k_cache[sc.blocks_flat]   # [TOT*8, 16, 8, 128] fp32
    VB = v_cache[sc.blocks_flat]

    maps = []
    for c in range(NCORES):
        qr = slice(512 * c, 512 * (c + 1))
        kr = slice(Q_SIZE + 128 * c, Q_SIZE + 128 * (c + 1))
        vr = slice(Q_SIZE + KV_SIZE + 128 * c, Q_SIZE + KV_SIZE + 128 * (c + 1))
        wq_sh = np.concatenate([W_qkv[qr], W_qkv[kr], W_qkv[vr]], axis=0)  # [768, 4096]
        wq_in = np.ascontiguousarray(
            wq_sh.T.astype(BF16).reshape(32, 128, 768).transpose(1, 0, 2))
        bq_sh = np.concatenate([b_qkv[qr], b_qkv[kr], b_qkv[vr]])
        bq_in = bq_sh[None, :].astype(BF16)
        wo_in = np.ascontiguousarray(
            W_o[:, qr].T.astype(BF16).reshape(4, 128, HIDDEN).transpose(1, 0, 2))

        # K: [TOT, 128 tok, 128 D] -> [128 D, TOT*128] fp8
        kc = KB[:, :, c, :].astype(FP8KV).reshape(TOT, CHUNK, D)
        kg_in = np.ascontiguousarray(
            kc.transpose(2, 0, 1).reshape(D, TOT * CHUNK))

        # V split: hi dims 0:VBF bf16, lo dims VBF:128 + validity fp8
        vc = VB[:, :, c, :].reshape(TOT, CHUNK, D) * sc.valid[:, :, None]
        vh_in = np.ascontiguousarray(
            vc[:, :, :VBF].astype(BF16).transpose(1, 0, 2).reshape(
                CHUNK, TOT * VBF))
        vlo = np.zeros((TOT, CHUNK, VLW), np.float32)
        vlo[:, :, :D - VBF] = vc[:, :, VBF:]
        vlo[:, :, D - VBF] = sc.valid
        vl_in = np.ascontiguousarray(
            vlo.astype(FP8KV).transpose(1, 0, 2).reshape(CHUNK, TOT * VLW))

        maps.append({
            "ht": ht_in, "wq": wq_in, "wo": wo_in, "bq": bq_in,
            "trig": np.concatenate([sc.cosf, sc.sinf], axis=1),
            "mdiag": sc.mdiag,
            "kg": kg_in, "vh": vh_in, "vl": vl_in,
        })
    return maps


_TRACE = {"on": False, "result": None}


def kernel(hidden_states, W_qkv, b_qkv, W_o, b_o, k_cache, v_cache,
           block_tables, context_lens):
    import concourse.tile as tile
    import concourse.mybir as mybir
    from concourse import bacc
    from concourse.bass_utils import run_bass_kernel_spmd

    sched = _Schedule(context_lens, block_tables)
    nc = bacc.Bacc("TRN2", target_bir_lowering=False, debug=False)
    _emit(nc, tile, mybir, sched)

    in_maps = _build_inputs(sched, np.asarray(hidden_states, np.float32),
                            np.asarray(W_qkv, np.float32),
                            np.asarray(b_qkv, np.float32),
                            np.asarray(W_o, np.float32),
                            np.asarray(k_cache, np.float32),
                            np.asarray(v_cache, np.float32))

    res = run_bass_kernel_spmd(nc, in_maps, core_ids=list(range(NCORES)),
                               trace=_TRACE["on"])
    _TRACE["result"] = res

    acc = np.zeros((B, HIDDEN), np.float64)
    for c in range(NCORES):
        o128 = res.results[c]["out"].astype(np.float64)  # [128, 1024]
        # o128[p, 32*j + s] = out[s, 128*j + p]
        acc += o128.reshape(128, 32, 32).transpose(2, 1, 0).reshape(B, HIDDEN)
    acc += np.asarray(b_o, np.float64)[None, :]
    return acc.astype(np.float32)
